# revision 18
# baseline (speedup 1.0000x reference)
"""Trainium2 Bass kernel for nn_CholecFixScore (pairwise-IoU mask scoring).

Math (per sample n):
    Gp (P=16, HW) and Gt (T=8, HW) are binary {0,1} masks.
    inters[p,t] = sum_hw Gp[p]*Gt[t];  sp[p] = sum Gp[p];  st[t] = sum Gt[t]
    iou = inters / max(sp+st-inters, 1)            (union==0 => inters==0 => iou 0)
    w[p] = max_t iou[p,t]
    den[hw] = sum_p Gp[p,hw];  r = 1/max(den,1)    (den==0 pixels have Gp==0)
    score[n] = (1/HW) * sum_p w[p] * S[p],  S[p] = sum_hw Gp[p,hw]*r[hw]
which equals the reference's mean over pixels of (sum_p w[p]Gp[p,hw])/den[hw].

Sharding: pure data parallel, 2 samples per core on 8 cores.

Host-side packing (free wrt HW time): masks are {0,1}, EXACT in fp8e4m3,
which halves HBM traffic again vs bf16 and enables DoubleRow matmuls.
Pixel index hw = k*392 + j with k = SBUF partition (128), j in [0,392).
j is chunked as j = c*7 + js  (c in [0,56), js in [0,7)).
  gpw[s] (128, 56*7*17) fp8, free = (c, js, p'):  p' = 16 Gp masks | ones row
  gte[s] (128, 56*77)  fp8, free = (c, x): x = (js,u) u = 8 Gt | ones (63 cols)
                                         then (js, r_hi|r_lo)     (14 cols)
The ones ROW (p'=16) makes the main GEMM emit rhs column sums == st[t];
the ones COLUMN (u=8) emits sp[p].  r is computed on-chip and written as
an exact-ish two-term fp8 split r = r_hi + r_lo (rel err ~2^-8 << 2e-2).

Main GEMM per sample: 28 DoubleRow fp8 matmuls (2 j-chunks each):
  lhsT = gpw view (128, 2, 119), rhs = gte view (128, 2, 77), contiguous
  psum (119, 77) accumulates.  Valid outputs are the 7 diagonal js-blocks;
eye-selector matmuls relocate+sum them into acc (17, 11) =
  [inters|sp|S_hi|S_lo ; st|...].

den'[k,j] = sum_p' gpw (ALL 17, contiguous innermost reduce) = den+1;
den = max(den'-1, 1) fused in one dual-op tensor_scalar; 1/den via
reciprocal_approx_fast.  All DMAs are plain HWDGE with 128 contiguous
descriptors, split over BOTH hwdge queues (sync + scalar) because one
queue issues only ~1.5 DMA/us.  Dummy matmuls reading each arrived gpw
DMA piece keep the PE clock at 2.4 GHz through the DMA phase.
"""

import numpy as np
import ml_dtypes

import concourse.bass as bass
import concourse.tile as tile
from concourse import mybir
from concourse.bass_utils import run_bass_kernel_spmd

F32 = mybir.dt.float32
BF16 = mybir.dt.bfloat16
FP8 = mybir.dt.float8e4
ADD = mybir.AluOpType.add
NPF8 = ml_dtypes.float8_e4m3fn

N, P, T = 16, 16, 8
H, W = 224, 224
HW = H * W            # 50176
PART = 128
JW = HW // PART       # 392 j values per partition
JS = 7                # j values per chunk
NCH = JW // JS        # 56 chunks
NPAIR = NCH // 2      # 28 DoubleRow chunk pairs
PP = P + 1            # 16 masks + ones row
UA = T + 1            # u-part: 8 Gt | ones
MCH = JS * PP         # 119 lhsT cols per chunk
UCOLS = JS * UA       # 63 u-part cols per chunk
RCOLS = JS * 2        # 14 r cols per chunk (js-major, hi|lo)
XCH = UCOLS + RCOLS   # 77 rhs cols per chunk
GPW_COLS = NCH * MCH  # 6664
GTE_COLS = NCH * XCH  # 4312
NCORES = 8
SPC = N // NCORES     # samples per core = 2
INV_HW = 1.0 / HW
USE_DR = False        # DoubleRow perf mode (fp8, 2 k-tiles per matmul)
# DMA pieces: gpw halves, gte thirds (chunk-pair aligned: 20|18|18 chunks)
GTE_CUTS = [0, 20, 38, NCH]
N_WARM_PER_PIECE = 5  # dummy matmuls gated on each gpw DMA piece


def _split_multi_waits(nc):
    """The pinned walrus encodes only ONE sync-wait per instruction; split
    Tile-emitted multi-wait instructions into single-wait NOPs ahead of them
    (same engine, program order => identical semantics)."""
    n = 0
    for f in nc.m.functions:
        for bb in f.blocks:
            insts = bb.instructions
            newlist = []
            changed = False
            for ins in insts:
                si = ins.sync_info
                if si is not None and si.on_wait is not None and len(si.on_wait) > 1:
                    waits = list(si.on_wait)
                    for w in waits[:-1]:
                        n += 1
                        newlist.append(
                            mybir.InstNoOp(
                                name=f"I-waitsplit-{n}",
                                engine=ins.engine,
                                ins=[],
                                outs=[],
                                sync_info=mybir.SyncInfo(on_wait=[w], on_update=[]),
                            )
                        )
                    ins.sync_info = mybir.SyncInfo(
                        on_wait=[waits[-1]], on_update=list(si.on_update or [])
                    )
                    changed = True
                newlist.append(ins)
            if changed:
                while len(insts):
                    insts.pop()
                for x in newlist:
                    insts.append(x)
    return n


def _build():
    nc = bass.Bass("TRN2", target_bir_lowering=False, debug=False)
    gpw = nc.dram_tensor("gpw", [SPC, PART, GPW_COLS], FP8, kind="ExternalInput")
    gte = nc.dram_tensor("gte", [SPC, PART, GTE_COLS], FP8, kind="ExternalInput")
    # ce = [ eye(128) | sel16 | ones16 ]:
    #   sel16[k, m] = 1 iff k == P  (broadcasts acc's st row to partitions 0..15)
    #   ones16[k] = 1 iff k < 16    (final score reduction column)
    ce = nc.dram_tensor("ce", [PART, PART + 17], F32, kind="ExternalInput")
    y = nc.dram_tensor("y", [1, SPC], F32, kind="ExternalOutput")

    with tile.TileContext(nc) as tc:
        with (
            tc.tile_pool(name="big", bufs=2) as big,
            tc.tile_pool(name="scratch", bufs=2) as scratch,
            tc.tile_pool(name="small", bufs=2) as small,
            tc.tile_pool(name="singles", bufs=1) as singles,
            tc.tile_pool(name="psmain", bufs=2, space="PSUM") as psmain,
            tc.tile_pool(name="pswarm", bufs=1, space="PSUM") as pswarm,
            tc.tile_pool(name="psaux", bufs=1, space="PSUM") as psaux,
        ):
            e_sb = singles.tile([PART, PART + 17], F32)
            out_sb = singles.tile([1, SPC], F32)

            gpws = [big.tile([PART, GPW_COLS], FP8, tag="gpw", name=f"gpw{s}")
                    for s in range(SPC)]
            gtes = [big.tile([PART, GTE_COLS], FP8, tag="gte", name=f"gte{s}")
                    for s in range(SPC)]

            # ---- input DMAs: plain HWDGE, 128 contiguous descriptors each,
            # split across both hwdge queues (sync issues ~650ns/DMA).
            # Arrival order: gpw0, gpw1 (halves in parallel on the 2 queues),
            # then gte pieces interleaved s0/s1 so both main GEMMs chase the
            # tail of the DMA stream. ----
            nc.scalar.dma_start(out=e_sb[:, :], in_=ce[:, :])
            # gpw0 quartered (2 per queue) so den0 pipelines behind arrival;
            # gpw1/gte halved across the queues.  12 DMAs total; early ones
            # complete before their semaphore lanes recycle.
            GQ = GPW_COLS // 4
            for q in range(4):
                eng = nc.sync if q % 2 == 0 else nc.scalar
                eng.dma_start(
                    out=gpws[0][:, q * GQ : (q + 1) * GQ],
                    in_=gpw[0, :, q * GQ : (q + 1) * GQ],
                )
            GH = GPW_COLS // 2
            GTH = (GTE_CUTS[1]) * XCH  # piece 1 boundary (20 chunks)
            nc.sync.dma_start(out=gtes[0][:, 0:GTH], in_=gte[0, :, 0:GTH])
            nc.scalar.dma_start(
                out=gtes[0][:, GTH:GTE_COLS], in_=gte[0, :, GTH:GTE_COLS]
            )
            nc.sync.dma_start(out=gpws[1][:, 0:GH], in_=gpw[1, :, 0:GH])
            nc.scalar.dma_start(
                out=gpws[1][:, GH:GPW_COLS], in_=gpw[1, :, GH:GPW_COLS]
            )
            nc.sync.dma_start(out=gtes[1][:, 0:GTH], in_=gte[1, :, 0:GTH])
            nc.scalar.dma_start(
                out=gtes[1][:, GTH:GTE_COLS], in_=gte[1, :, GTH:GTE_COLS]
            )

            # ---- PE warmup: HAM releases the clock gate after ~3.4us of
            # sustained activity.  Gate dummy matmuls on the arriving gpw
            # pieces so the array stays busy through the whole DMA phase and
            # the main GEMMs run at 2.4 GHz.  Results are never read. ----
            warm_ps = pswarm.tile([MCH, 512], F32)
            for q in range(4):
                for k in range(3):
                    base = q * GQ
                    nc.tensor.matmul(
                        warm_ps[:, :],
                        gpws[0][:, base : base + MCH],
                        gpws[0][:, base + MCH + k * 512 : base + MCH + (k + 1) * 512],
                    )
            for half in range(2):
                for k in range(2):
                    base = half * GH
                    nc.tensor.matmul(
                        warm_ps[:, :],
                        gpws[1][:, base : base + MCH],
                        gpws[1][:, base + MCH + k * 512 : base + MCH + (k + 1) * 512],
                    )

            # ---- den/r per sample (DVE): den' = sum over ALL 17 p' (incl
            # ones row) = den+1, contiguous innermost reduce, one per gpw
            # half; den = max(den'-1, 1) fused; r = 1/den approx (fp32);
            # two-term fp8 split r = r_hi + r_lo written into gte r slots. ----
            rparts = {}

            def den_r(s):
                denp = scratch.tile([PART, JW], BF16, tag="denp", name=f"denp{s}")
                gv = gpws[s][:].rearrange("part (cj p) -> part cj p", p=PP)
                HJ = JW // 2
                # all reduces on DVE (gpsimd only does cross-partition);
                # s0 quartered behind the quartered gpw0 DMA, s1 halved
                cuts = 4 if s == 0 else 2
                CJ = JW // cuts
                with nc.allow_low_precision(
                    reason="den is an integer <=17; exact in bf16"
                ):
                    for q in range(cuts):
                        nc.vector.tensor_reduce(
                            out=denp[:, q * CJ : (q + 1) * CJ],
                            in_=gv[:, q * CJ : (q + 1) * CJ, :],
                            axis=mybir.AxisListType.X,
                            op=ADD,
                        )
                den32 = scratch.tile([PART, JW], F32, tag="den32", name=f"den32{s}")
                lnv = scratch.tile([PART, JW], F32, tag="lnv", name=f"lnv{s}")
                r32 = scratch.tile([PART, JW], F32, tag="r32", name=f"r32{s}")
                hi8 = scratch.tile([PART, JW], FP8, tag="hi8", name=f"hi8{s}")
                lo32 = scratch.tile([PART, JW], F32, tag="lo32", name=f"lo32{s}")
                for h in range(2):
                    sl = slice(h * HJ, (h + 1) * HJ)
                    # per-half chain: DVE handles only s0h1; the rest on
                    # gpsimd so the DVE can keep reducing
                    eng = nc.vector if (s == 0 and h == 0) else nc.gpsimd
                    eng.tensor_scalar(
                        out=den32[:, sl],
                        in0=denp[:, sl],
                        scalar1=-1.0,
                        scalar2=1.0,
                        op0=ADD,
                        op1=mybir.AluOpType.max,
                    )
                    # 1/den = exp(-ln(den)) on the ACT engine (err ~1e-3)
                    nc.scalar.activation(
                        out=lnv[:, sl], in_=den32[:, sl],
                        func=mybir.ActivationFunctionType.Ln,
                    )
                    nc.scalar.activation(
                        out=r32[:, sl], in_=lnv[:, sl],
                        func=mybir.ActivationFunctionType.Exp, scale=-1.0,
                    )
                    with nc.allow_low_precision(
                        reason="two-term fp8 split of r: rel err ~2^-8 << 2e-2"
                    ):
                        eng.tensor_copy(hi8[:, sl], r32[:, sl])
                        eng.tensor_tensor(
                            lo32[:, sl], r32[:, sl], hi8[:, sl],
                            mybir.AluOpType.subtract,
                        )
                rparts[s] = (hi8, lo32)

            # r scatter into the gte r slots (after that piece's DMA landed)
            def scatter_r(s, piece):
                hi8, lo32 = rparts[s]
                clo, chi = GTE_CUTS[piece], GTE_CUTS[piece + 1]
                rv = gtes[s][:].rearrange("part (c x) -> part c x", x=XCH)[
                    :, clo:chi, UCOLS:XCH
                ].rearrange("part c (js two) -> part c js two", two=2)
                def jsrc(t):
                    return t[:].rearrange(
                        "part (c js one) -> part c js one", js=JS, one=1
                    )[:, clo:chi, :, :]
                eng = nc.vector if s == 0 else nc.gpsimd
                with nc.allow_low_precision(reason="fp8 r split scatter"):
                    eng.tensor_copy(rv[:, :, :, 0:1], jsrc(hi8))
                    eng.tensor_copy(rv[:, :, :, 1:2], jsrc(lo32))

            # ---- main GEMM: DoubleRow fp8, 2 j-chunks per matmul, issued
            # piece-by-piece chasing the gte DMA stream ----
            pss = {
                s: psmain.tile([MCH, XCH], F32, tag="main", name=f"ps_main{s}")
                for s in range(SPC)
            }

            def main_piece(s, piece):
                lo2, hi2 = GTE_CUTS[piece] // 2, GTE_CUTS[piece + 1] // 2
                gpv = gpws[s][:].rearrange(
                    "part (c2 two m) -> part c2 two m", two=2, m=MCH
                )
                gtv = gtes[s][:].rearrange(
                    "part (c2 two x) -> part c2 two x", two=2, x=XCH
                )
                for c2 in range(lo2, hi2):
                    if USE_DR:
                        nc.tensor.matmul(
                            pss[s][:, :],
                            gpv[:, c2],
                            gtv[:, c2],
                            start=(c2 == 0),
                            stop=(c2 == NPAIR - 1),
                            perf_mode=mybir.MatmulPerfMode.DoubleRow,
                        )
                    else:
                        for c in (2 * c2, 2 * c2 + 1):
                            nc.tensor.matmul(
                                pss[s][:, :],
                                gpws[s][:, c * MCH : (c + 1) * MCH],
                                gtes[s][:, c * XCH : (c + 1) * XCH],
                                start=(c == 0),
                                stop=(c == NCH - 1),
                            )

            accs = {}

            def extract(s):
                ext = small.tile([MCH, XCH], F32, tag="ext", name=f"ext{s}")
                nc.vector.tensor_copy(ext[:, :], pss[s][:, :])
                ps_acc = psaux.tile([PP, UA + 2], F32, tag="acc", name=f"ps_acc{s}")
                for js in range(JS):
                    nc.tensor.matmul(
                        ps_acc[:, 0:UA],
                        e_sb[0:MCH, js * PP : (js + 1) * PP],
                        ext[:, js * UA : (js + 1) * UA],
                        start=(js == 0),
                        stop=(js == JS - 1),
                    )
                for js in range(JS):
                    nc.tensor.matmul(
                        ps_acc[:, UA : UA + 2],
                        e_sb[0:MCH, js * PP : (js + 1) * PP],
                        ext[:, UCOLS + js * 2 : UCOLS + js * 2 + 2],
                        start=(js == 0),
                        stop=(js == JS - 1),
                    )
                acc = small.tile([PP, UA + 2], F32, tag="accsb", name=f"acc{s}")
                nc.vector.tensor_copy(acc[:, :], ps_acc[:, :])
                accs[s] = acc

            def finish(s):
                acc = accs[s]
                # broadcast st (ones row of acc) to 16 partitions via sel16
                ps_st16 = psaux.tile([16, T], F32, tag="st16", name=f"ps_st16{s}")
                nc.tensor.matmul(
                    ps_st16[:, :], e_sb[0:PP, PART : PART + 16], acc[0:PP, 0:T]
                )
                # unions = max((st16 + sp) - inters, 1);  iou = inters/unions
                unions = small.tile([16, T], F32, tag=f"un{s}", name=f"unions{s}")
                nc.vector.scalar_tensor_tensor(
                    out=unions[:, :],
                    in0=ps_st16[:, :],
                    scalar=acc[0:P, T : T + 1],
                    in1=acc[0:P, 0:T],
                    op0=ADD,
                    op1=mybir.AluOpType.subtract,
                )
                nc.vector.tensor_scalar_max(
                    out=unions[:, :], in0=unions[:, :], scalar1=1.0
                )
                uln = small.tile([16, T], F32, tag=f"ul{s}", name=f"uln{s}")
                nc.scalar.activation(
                    out=uln[:, :], in_=unions[:, :],
                    func=mybir.ActivationFunctionType.Ln,
                )
                uinv = small.tile([16, T], F32, tag=f"ui{s}", name=f"uinv{s}")
                nc.scalar.activation(
                    out=uinv[:, :], in_=uln[:, :],
                    func=mybir.ActivationFunctionType.Exp, scale=-1.0,
                )
                iou = small.tile([16, T], F32, tag=f"iou{s}", name=f"iou{s}")
                nc.vector.tensor_tensor(
                    iou[:, :], acc[0:P, 0:T], uinv[:, :], mybir.AluOpType.mult
                )
                wmax = small.tile([16, 1], F32, tag=f"wm{s}", name=f"wmax{s}")
                nc.vector.tensor_reduce(
                    out=wmax[:, :],
                    in_=iou[:, :],
                    axis=mybir.AxisListType.X,
                    op=mybir.AluOpType.max,
                )
                # ws = (S_hi + S_lo) * w
                ws = small.tile([16, 1], F32, tag=f"ws{s}", name=f"ws{s}")
                nc.vector.scalar_tensor_tensor(
                    out=ws[:, :],
                    in0=acc[0:P, UA : UA + 1],
                    scalar=acc[0:P, UA + 1 : UA + 2],
                    in1=wmax[:, :],
                    op0=ADD,
                    op1=mybir.AluOpType.mult,
                )
                ps_score = psaux.tile([1, 1], F32, tag="sc", name=f"ps_score{s}")
                nc.tensor.matmul(
                    ps_score[:, :], e_sb[0:16, PART + 16 : PART + 17], ws[:, :]
                )
                nc.vector.tensor_scalar_mul(
                    out=out_sb[0:1, s : s + 1], in0=ps_score[:, :], scalar1=INV_HW
                )

            den_r(0)
            for piece in range(len(GTE_CUTS) - 1):
                scatter_r(0, piece)
            den_r(1)
            for piece in range(len(GTE_CUTS) - 1):
                main_piece(0, piece)
            for piece in range(len(GTE_CUTS) - 1):
                scatter_r(1, piece)
            extract(0)
            for piece in range(len(GTE_CUTS) - 1):
                main_piece(1, piece)
            extract(1)
            finish(0)
            finish(1)

            nc.sync.dma_start(out=y[:, :], in_=out_sb[:, :])

    _split_multi_waits(nc)
    return nc


_NC = None


def _get_nc():
    global _NC
    if _NC is None:
        _NC = _build()
    return _NC


def _pack(groups_pred: np.ndarray, groups_true: np.ndarray):
    # binarize (match torch .bool(): nonzero -> 1); {0,1} exact in fp8e4m3
    gp = (groups_pred != 0).astype(NPF8)     # (N, P, H, W)
    gt = (groups_true != 0).astype(NPF8)     # (N, T, H, W)
    # (N, P, PART, NCH, JS) -> (N, PART, NCH, JS, P) + ones row
    gp5 = gp.reshape(N, P, PART, NCH, JS).transpose(0, 2, 3, 4, 1)
    gpw = np.empty((N, PART, NCH, JS, PP), dtype=NPF8)
    gpw[..., 0:P] = gp5
    gpw[..., P] = NPF8(1.0)
    gt5 = gt.reshape(N, T, PART, NCH, JS).transpose(0, 2, 3, 4, 1)
    gte = np.zeros((N, PART, NCH, XCH), dtype=NPF8)
    upart = np.empty((N, PART, NCH, JS, UA), dtype=NPF8)
    upart[..., 0:T] = gt5
    upart[..., T] = NPF8(1.0)
    gte[..., 0:UCOLS] = upart.reshape(N, PART, NCH, UCOLS)
    gpw = np.ascontiguousarray(gpw.reshape(NCORES, SPC, PART, GPW_COLS))
    gte = np.ascontiguousarray(gte.reshape(NCORES, SPC, PART, GTE_COLS))
    return gpw, gte


def make_in_maps(groups_pred: np.ndarray, groups_true: np.ndarray) -> list[dict]:
    gpw, gte = _pack(groups_pred, groups_true)
    ce = np.zeros((PART, PART + 17), dtype=np.float32)
    ce[:, 0:PART] = np.eye(PART, dtype=np.float32)
    ce[P, PART : PART + 16] = 1.0
    ce[0:16, PART + 16] = 1.0
    return [{"gpw": gpw[c], "gte": gte[c], "ce": ce} for c in range(NCORES)]


def kernel(groups_pred: np.ndarray, groups_true: np.ndarray) -> np.ndarray:
    assert groups_pred.shape == (N, P, H, W)
    assert groups_true.shape == (N, T, H, W)
    in_maps = make_in_maps(groups_pred, groups_true)
    res = run_bass_kernel_spmd(_get_nc(), in_maps, core_ids=list(range(NCORES)))
    out = np.empty((N,), dtype=np.float32)
    for c in range(NCORES):
        out[c * SPC : (c + 1) * SPC] = res.results[c]["y"][0]
    return out


# revision 22
# speedup vs baseline: 1.0439x; 1.0439x over previous
"""Trainium2 Bass kernel for nn_CholecFixScore (pairwise-IoU mask scoring).

Math (per sample n):
    Gp (P=16, HW) and Gt (T=8, HW) are binary {0,1} masks.
    inters[p,t] = sum_hw Gp[p]*Gt[t];  sp[p] = sum Gp[p];  st[t] = sum Gt[t]
    iou = inters / max(sp+st-inters, 1)            (union==0 => inters==0 => iou 0)
    w[p] = max_t iou[p,t]
    den[hw] = sum_p Gp[p,hw];  r = 1/max(den,1)    (den==0 pixels have Gp==0)
    score[n] = (1/HW) * sum_p w[p] * S[p],  S[p] = sum_hw Gp[p,hw]*r[hw]
which equals the reference's mean over pixels of (sum_p w[p]Gp[p,hw])/den[hw].

Sharding: pure data parallel, 2 samples per core on 8 cores.

Host-side packing (free wrt HW time): masks are {0,1}, EXACT in fp8e4m3,
which halves HBM traffic again vs bf16 and enables DoubleRow matmuls.
Pixel index hw = k*392 + j with k = SBUF partition (128), j in [0,392).
j is chunked as j = c*7 + js  (c in [0,56), js in [0,7)).
  gpw[s] (128, 56*7*17) fp8, free = (c, js, p'):  p' = 16 Gp masks | ones row
  gte[s] (128, 56*77)  fp8, free = (c, x): x = (js,u) u = 8 Gt | ones (63 cols)
                                         then (js, r_hi|r_lo)     (14 cols)
The ones ROW (p'=16) makes the main GEMM emit rhs column sums == st[t];
the ones COLUMN (u=8) emits sp[p].  r is computed on-chip and written as
an exact-ish two-term fp8 split r = r_hi + r_lo (rel err ~2^-8 << 2e-2).

Main GEMM per sample: 28 DoubleRow fp8 matmuls (2 j-chunks each):
  lhsT = gpw view (128, 2, 119), rhs = gte view (128, 2, 77), contiguous
  psum (119, 77) accumulates.  Valid outputs are the 7 diagonal js-blocks;
eye-selector matmuls relocate+sum them into acc (17, 11) =
  [inters|sp|S_hi|S_lo ; st|...].

den'[k,j] = sum_p' gpw (ALL 17, contiguous innermost reduce) = den+1;
den = max(den'-1, 1) fused in one dual-op tensor_scalar; 1/den via
reciprocal_approx_fast.  All DMAs are plain HWDGE with 128 contiguous
descriptors, split over BOTH hwdge queues (sync + scalar) because one
queue issues only ~1.5 DMA/us.  Dummy matmuls reading each arrived gpw
DMA piece keep the PE clock at 2.4 GHz through the DMA phase.
"""

import numpy as np
import ml_dtypes

import concourse.bass as bass
import concourse.tile as tile
from concourse import mybir
from concourse.bass_utils import run_bass_kernel_spmd

F32 = mybir.dt.float32
BF16 = mybir.dt.bfloat16
FP8 = mybir.dt.float8e4
ADD = mybir.AluOpType.add
NPF8 = ml_dtypes.float8_e4m3fn

N, P, T = 16, 16, 8
H, W = 224, 224
HW = H * W            # 50176
PART = 128
JW = HW // PART       # 392 j values per partition
JS = 7                # j values per chunk
NCH = JW // JS        # 56 chunks
NPAIR = NCH // 2      # 28 DoubleRow chunk pairs
PP = P + 1            # 16 masks + ones row
UA = T + 1            # u-part: 8 Gt | ones
MCH = JS * PP         # 119 lhsT cols per chunk
UCOLS = JS * UA       # 63 u-part cols per chunk
RCOLS = JS * 2        # 14 r cols per chunk (js-major, hi|lo)
XCH = UCOLS + RCOLS   # 77 rhs cols per chunk
GPW_COLS = NCH * MCH  # 6664
GTE_COLS = NCH * XCH  # 4312
NCORES = 8
SPC = N // NCORES     # samples per core = 2
INV_HW = 1.0 / HW
USE_DR = False        # DoubleRow perf mode (fp8, 2 k-tiles per matmul)
# DMA pieces: gpw halves, gte thirds (chunk-pair aligned: 20|18|18 chunks)
GTE_CUTS = [0, 20, 38, NCH]
N_WARM_PER_PIECE = 5  # dummy matmuls gated on each gpw DMA piece


def _split_multi_waits(nc):
    """The pinned walrus encodes only ONE sync-wait per instruction; split
    Tile-emitted multi-wait instructions into single-wait NOPs ahead of them
    (same engine, program order => identical semantics)."""
    n = 0
    for f in nc.m.functions:
        for bb in f.blocks:
            insts = bb.instructions
            newlist = []
            changed = False
            for ins in insts:
                si = ins.sync_info
                if si is not None and si.on_wait is not None and len(si.on_wait) > 1:
                    waits = list(si.on_wait)
                    for w in waits[:-1]:
                        n += 1
                        newlist.append(
                            mybir.InstNoOp(
                                name=f"I-waitsplit-{n}",
                                engine=ins.engine,
                                ins=[],
                                outs=[],
                                sync_info=mybir.SyncInfo(on_wait=[w], on_update=[]),
                            )
                        )
                    ins.sync_info = mybir.SyncInfo(
                        on_wait=[waits[-1]], on_update=list(si.on_update or [])
                    )
                    changed = True
                newlist.append(ins)
            if changed:
                while len(insts):
                    insts.pop()
                for x in newlist:
                    insts.append(x)
    return n


def _build():
    nc = bass.Bass("TRN2", target_bir_lowering=False, debug=False)
    gpw = nc.dram_tensor("gpw", [SPC, PART, GPW_COLS], FP8, kind="ExternalInput")
    gte = nc.dram_tensor("gte", [SPC, PART, GTE_COLS], FP8, kind="ExternalInput")
    # ce = [ eye(128) | sel16 | ones16 ]:
    #   sel16[k, m] = 1 iff k == P  (broadcasts acc's st row to partitions 0..15)
    #   ones16[k] = 1 iff k < 16    (final score reduction column)
    ce = nc.dram_tensor("ce", [PART, PART + 17], F32, kind="ExternalInput")
    y = nc.dram_tensor("y", [1, SPC], F32, kind="ExternalOutput")

    with tile.TileContext(nc) as tc:
        with (
            tc.tile_pool(name="big", bufs=2) as big,
            tc.tile_pool(name="scratch", bufs=2) as scratch,
            tc.tile_pool(name="small", bufs=2) as small,
            tc.tile_pool(name="singles", bufs=1) as singles,
            tc.tile_pool(name="psmain", bufs=2, space="PSUM") as psmain,
            tc.tile_pool(name="pswarm", bufs=1, space="PSUM") as pswarm,
            tc.tile_pool(name="psaux", bufs=1, space="PSUM") as psaux,
        ):
            e_sb = singles.tile([PART, PART + 17], F32)
            out_sb = singles.tile([1, SPC], F32)
            lnbias = singles.tile([PART, 1], F32)

            gpws = [big.tile([PART, GPW_COLS], FP8, tag="gpw", name=f"gpw{s}")
                    for s in range(SPC)]
            gtes = [big.tile([PART, GTE_COLS], FP8, tag="gte", name=f"gte{s}")
                    for s in range(SPC)]

            # ---- input DMAs: plain HWDGE, 128 contiguous descriptors each,
            # split across both hwdge queues (sync issues ~650ns/DMA).
            # Arrival order: gpw0, gpw1 (halves in parallel on the 2 queues),
            # then gte pieces interleaved s0/s1 so both main GEMMs chase the
            # tail of the DMA stream. ----
            # gpw0 quartered (2 per queue) so den0 pipelines behind arrival;
            # gpw1/gte halved across the queues.  12 DMAs total; early ones
            # complete before their semaphore lanes recycle.  high_priority
            # hoists the first DMAs ahead of the engines' register init.
            GQ = GPW_COLS // 4
            with tc.high_priority():
                nc.gpsimd.memset(lnbias[:, :], float(2.0 ** -14 - 1.0))
                nc.scalar.dma_start(out=e_sb[:, :], in_=ce[:, :])
                for q in range(4):
                    eng = nc.sync if q % 2 == 0 else nc.scalar
                    eng.dma_start(
                        out=gpws[0][:, q * GQ : (q + 1) * GQ],
                        in_=gpw[0, :, q * GQ : (q + 1) * GQ],
                    )
            GH = GPW_COLS // 2
            GTH = (GTE_CUTS[1]) * XCH  # piece 1 boundary (20 chunks)
            nc.sync.dma_start(out=gtes[0][:, 0:GTH], in_=gte[0, :, 0:GTH])
            nc.scalar.dma_start(
                out=gtes[0][:, GTH:GTE_COLS], in_=gte[0, :, GTH:GTE_COLS]
            )
            nc.sync.dma_start(out=gpws[1][:, 0:GH], in_=gpw[1, :, 0:GH])
            nc.scalar.dma_start(
                out=gpws[1][:, GH:GPW_COLS], in_=gpw[1, :, GH:GPW_COLS]
            )
            nc.sync.dma_start(out=gtes[1][:, 0:GTH], in_=gte[1, :, 0:GTH])
            nc.scalar.dma_start(
                out=gtes[1][:, GTH:GTE_COLS], in_=gte[1, :, GTH:GTE_COLS]
            )

            # ---- PE warmup: HAM releases the clock gate after ~3.4us of
            # sustained activity.  Gate dummy matmuls on the arriving gpw
            # pieces so the array stays busy through the whole DMA phase and
            # the main GEMMs run at 2.4 GHz.  Results are never read. ----
            warm_ps = pswarm.tile([MCH, 512], F32)
            for q in range(4):
                for k in range(3):
                    base = q * GQ
                    nc.tensor.matmul(
                        warm_ps[:, :],
                        gpws[0][:, base : base + MCH],
                        gpws[0][:, base + MCH + k * 512 : base + MCH + (k + 1) * 512],
                    )
            for half in range(2):
                for k in range(2):
                    base = half * GH
                    nc.tensor.matmul(
                        warm_ps[:, :],
                        gpws[1][:, base : base + MCH],
                        gpws[1][:, base + MCH + k * 512 : base + MCH + (k + 1) * 512],
                    )

            # ---- den/r per sample (DVE): den' = sum over ALL 17 p' (incl
            # ones row) = den+1, contiguous innermost reduce, one per gpw
            # half; den = max(den'-1, 1) fused; r = 1/den approx (fp32);
            # two-term fp8 split r = r_hi + r_lo written into gte r slots. ----
            rparts = {}
            denps = {}

            def den_reduce(s, frm, to):
                # DVE: den'[k,(c,js)] = sum over all 17 p' (contiguous
                # innermost) = den+1, for j in [frm, to)
                if s not in denps:
                    denps[s] = scratch.tile(
                        [PART, JW], BF16, tag="denp", name=f"denp{s}"
                    )
                gv = gpws[s][:].rearrange("part (cj p) -> part cj p", p=PP)
                with nc.allow_low_precision(
                    reason="den is an integer <=17; exact in bf16"
                ):
                    nc.vector.tensor_reduce(
                        out=denps[s][:, frm:to],
                        in_=gv[:, frm:to, :],
                        axis=mybir.AxisListType.X,
                        op=ADD,
                    )

            def r_chain(s, h):
                # r = 1/max(den'-1, 1) via exp(-ln(den'-1+2^-14)) on the ACT
                # engine: where den=0 this gives a finite junk value (2^14)
                # that only ever multiplies Gp=0 in the GEMM.  The fp8
                # two-term split runs on gpsimd (single-op calls only; its
                # dual-op tensor_scalar is ~5x slower).
                if s not in rparts:
                    rparts[s] = (
                        scratch.tile([PART, JW], FP8, tag="hi8", name=f"hi8{s}"),
                        scratch.tile([PART, JW], F32, tag="lo32", name=f"lo32{s}"),
                        scratch.tile([PART, JW], F32, tag="lnv", name=f"lnv{s}"),
                        scratch.tile([PART, JW], F32, tag="r32", name=f"r32{s}"),
                        scratch.tile([PART, JW], F32, tag="den32", name=f"den32{s}"),
                    )
                hi8, lo32, lnv, r32 = rparts[s][0:4]
                HJ = JW // 2
                sl = slice(h * HJ, (h + 1) * HJ)
                den32 = rparts[s][4]
                nc.vector.tensor_scalar(
                    out=den32[:, sl],
                    in0=denps[s][:, sl],
                    scalar1=-1.0,
                    scalar2=1.0,
                    op0=ADD,
                    op1=mybir.AluOpType.max,
                )
                nc.scalar.activation(
                    out=lnv[:, sl], in_=den32[:, sl],
                    func=mybir.ActivationFunctionType.Ln,
                )
                nc.scalar.activation(
                    out=r32[:, sl], in_=lnv[:, sl],
                    func=mybir.ActivationFunctionType.Exp, scale=-1.0,
                )
                with nc.allow_low_precision(
                    reason="two-term fp8 split of r: rel err ~2^-8 << 2e-2"
                ):
                    nc.scalar.copy(out=hi8[:, sl], in_=r32[:, sl])
                    nc.gpsimd.tensor_tensor(
                        lo32[:, sl], r32[:, sl], hi8[:, sl],
                        mybir.AluOpType.subtract,
                    )

            # r scatter into the gte r slots (after that piece's DMA landed)
            def scatter_r(s, piece):
                hi8, lo32 = rparts[s][0], rparts[s][1]
                clo, chi = GTE_CUTS[piece], GTE_CUTS[piece + 1]
                rv = gtes[s][:].rearrange("part (c x) -> part c x", x=XCH)[
                    :, clo:chi, UCOLS:XCH
                ].rearrange("part c (js two) -> part c js two", two=2)
                def jsrc(t):
                    return t[:].rearrange(
                        "part (c js one) -> part c js one", js=JS, one=1
                    )[:, clo:chi, :, :]
                with nc.allow_low_precision(reason="fp8 r split scatter"):
                    nc.gpsimd.tensor_copy(rv[:, :, :, 0:1], jsrc(hi8))
                    nc.gpsimd.tensor_copy(rv[:, :, :, 1:2], jsrc(lo32))

            # ---- main GEMM: DoubleRow fp8, 2 j-chunks per matmul, issued
            # piece-by-piece chasing the gte DMA stream ----
            pss = {
                s: psmain.tile([MCH, XCH], F32, tag="main", name=f"ps_main{s}")
                for s in range(SPC)
            }

            def main_piece(s, piece):
                lo2, hi2 = GTE_CUTS[piece] // 2, GTE_CUTS[piece + 1] // 2
                gpv = gpws[s][:].rearrange(
                    "part (c2 two m) -> part c2 two m", two=2, m=MCH
                )
                gtv = gtes[s][:].rearrange(
                    "part (c2 two x) -> part c2 two x", two=2, x=XCH
                )
                for c2 in range(lo2, hi2):
                    if USE_DR:
                        nc.tensor.matmul(
                            pss[s][:, :],
                            gpv[:, c2],
                            gtv[:, c2],
                            start=(c2 == 0),
                            stop=(c2 == NPAIR - 1),
                            perf_mode=mybir.MatmulPerfMode.DoubleRow,
                        )
                    else:
                        for c in (2 * c2, 2 * c2 + 1):
                            nc.tensor.matmul(
                                pss[s][:, :],
                                gpws[s][:, c * MCH : (c + 1) * MCH],
                                gtes[s][:, c * XCH : (c + 1) * XCH],
                                start=(c == 0),
                                stop=(c == NCH - 1),
                            )

            accs = {}

            def extract(s):
                ext = small.tile([MCH, XCH], F32, tag="ext", name=f"ext{s}")
                nc.vector.tensor_copy(ext[:, :], pss[s][:, :])
                ps_acc = psaux.tile([PP, UA + 2], F32, tag="acc", name=f"ps_acc{s}")
                for js in range(JS):
                    nc.tensor.matmul(
                        ps_acc[:, 0:UA],
                        e_sb[0:MCH, js * PP : (js + 1) * PP],
                        ext[:, js * UA : (js + 1) * UA],
                        start=(js == 0),
                        stop=(js == JS - 1),
                    )
                for js in range(JS):
                    nc.tensor.matmul(
                        ps_acc[:, UA : UA + 2],
                        e_sb[0:MCH, js * PP : (js + 1) * PP],
                        ext[:, UCOLS + js * 2 : UCOLS + js * 2 + 2],
                        start=(js == 0),
                        stop=(js == JS - 1),
                    )
                acc = small.tile([PP, UA + 2], F32, tag="accsb", name=f"acc{s}")
                nc.vector.tensor_copy(acc[:, :], ps_acc[:, :])
                accs[s] = acc

            def finish(s):
                acc = accs[s]
                # broadcast st (ones row of acc) to 16 partitions via sel16
                ps_st16 = psaux.tile([16, T], F32, tag="st16", name=f"ps_st16{s}")
                nc.tensor.matmul(
                    ps_st16[:, :], e_sb[0:PP, PART : PART + 16], acc[0:PP, 0:T]
                )
                # unions = max((st16 + sp) - inters, 1);  iou = inters/unions
                unions = small.tile([16, T], F32, tag=f"un{s}", name=f"unions{s}")
                nc.vector.scalar_tensor_tensor(
                    out=unions[:, :],
                    in0=ps_st16[:, :],
                    scalar=acc[0:P, T : T + 1],
                    in1=acc[0:P, 0:T],
                    op0=ADD,
                    op1=mybir.AluOpType.subtract,
                )
                nc.vector.tensor_scalar_max(
                    out=unions[:, :], in0=unions[:, :], scalar1=1.0
                )
                uln = small.tile([16, T], F32, tag=f"ul{s}", name=f"uln{s}")
                nc.scalar.activation(
                    out=uln[:, :], in_=unions[:, :],
                    func=mybir.ActivationFunctionType.Ln,
                )
                uinv = small.tile([16, T], F32, tag=f"ui{s}", name=f"uinv{s}")
                nc.scalar.activation(
                    out=uinv[:, :], in_=uln[:, :],
                    func=mybir.ActivationFunctionType.Exp, scale=-1.0,
                )
                iou = small.tile([16, T], F32, tag=f"iou{s}", name=f"iou{s}")
                nc.vector.tensor_tensor(
                    iou[:, :], acc[0:P, 0:T], uinv[:, :], mybir.AluOpType.mult
                )
                wmax = small.tile([16, 1], F32, tag=f"wm{s}", name=f"wmax{s}")
                nc.vector.tensor_reduce(
                    out=wmax[:, :],
                    in_=iou[:, :],
                    axis=mybir.AxisListType.X,
                    op=mybir.AluOpType.max,
                )
                # ws = (S_hi + S_lo) * w
                ws = small.tile([16, 1], F32, tag=f"ws{s}", name=f"ws{s}")
                nc.vector.scalar_tensor_tensor(
                    out=ws[:, :],
                    in0=acc[0:P, UA : UA + 1],
                    scalar=acc[0:P, UA + 1 : UA + 2],
                    in1=wmax[:, :],
                    op0=ADD,
                    op1=mybir.AluOpType.mult,
                )
                ps_score = psaux.tile([1, 1], F32, tag="sc", name=f"ps_score{s}")
                nc.tensor.matmul(
                    ps_score[:, :], e_sb[0:16, PART + 16 : PART + 17], ws[:, :]
                )
                nc.vector.tensor_scalar_mul(
                    out=out_sb[0:1, s : s + 1], in0=ps_score[:, :], scalar1=INV_HW
                )

            QJ = JW // 4
            HJ = JW // 2
            # s0 half1 (quarters), chain, scatter piece0 -> main0 p0 early
            den_reduce(0, 0, QJ)
            den_reduce(0, QJ, 2 * QJ)
            r_chain(0, 0)
            scatter_r(0, 0)
            main_piece(0, 0)
            # s1 half1 -> main1 p0
            den_reduce(1, 0, HJ)
            r_chain(1, 0)
            scatter_r(1, 0)
            main_piece(1, 0)
            # s0 half2 -> main0 p1, p2
            den_reduce(0, 2 * QJ, 3 * QJ)
            den_reduce(0, 3 * QJ, JW)
            r_chain(0, 1)
            scatter_r(0, 1)
            scatter_r(0, 2)
            main_piece(0, 1)
            main_piece(0, 2)
            extract(0)
            # s1 half2 -> main1 p1, p2
            den_reduce(1, HJ, JW)
            r_chain(1, 1)
            scatter_r(1, 1)
            scatter_r(1, 2)
            main_piece(1, 1)
            main_piece(1, 2)
            extract(1)
            finish(0)
            finish(1)

            nc.sync.dma_start(out=y[:, :], in_=out_sb[:, :])

    _split_multi_waits(nc)
    return nc


_NC = None


def _get_nc():
    global _NC
    if _NC is None:
        _NC = _build()
    return _NC


def _pack(groups_pred: np.ndarray, groups_true: np.ndarray):
    # binarize (match torch .bool(): nonzero -> 1); {0,1} exact in fp8e4m3
    gp = (groups_pred != 0).astype(NPF8)     # (N, P, H, W)
    gt = (groups_true != 0).astype(NPF8)     # (N, T, H, W)
    # (N, P, PART, NCH, JS) -> (N, PART, NCH, JS, P) + ones row
    gp5 = gp.reshape(N, P, PART, NCH, JS).transpose(0, 2, 3, 4, 1)
    gpw = np.empty((N, PART, NCH, JS, PP), dtype=NPF8)
    gpw[..., 0:P] = gp5
    gpw[..., P] = NPF8(1.0)
    gt5 = gt.reshape(N, T, PART, NCH, JS).transpose(0, 2, 3, 4, 1)
    gte = np.zeros((N, PART, NCH, XCH), dtype=NPF8)
    upart = np.empty((N, PART, NCH, JS, UA), dtype=NPF8)
    upart[..., 0:T] = gt5
    upart[..., T] = NPF8(1.0)
    gte[..., 0:UCOLS] = upart.reshape(N, PART, NCH, UCOLS)
    gpw = np.ascontiguousarray(gpw.reshape(NCORES, SPC, PART, GPW_COLS))
    gte = np.ascontiguousarray(gte.reshape(NCORES, SPC, PART, GTE_COLS))
    return gpw, gte


def make_in_maps(groups_pred: np.ndarray, groups_true: np.ndarray) -> list[dict]:
    gpw, gte = _pack(groups_pred, groups_true)
    ce = np.zeros((PART, PART + 17), dtype=np.float32)
    ce[:, 0:PART] = np.eye(PART, dtype=np.float32)
    ce[P, PART : PART + 16] = 1.0
    ce[0:16, PART + 16] = 1.0
    return [{"gpw": gpw[c], "gte": gte[c], "ce": ce} for c in range(NCORES)]


def kernel(groups_pred: np.ndarray, groups_true: np.ndarray) -> np.ndarray:
    assert groups_pred.shape == (N, P, H, W)
    assert groups_true.shape == (N, T, H, W)
    in_maps = make_in_maps(groups_pred, groups_true)
    res = run_bass_kernel_spmd(_get_nc(), in_maps, core_ids=list(range(NCORES)))
    out = np.empty((N,), dtype=np.float32)
    for c in range(NCORES):
        out[c * SPC : (c + 1) * SPC] = res.results[c]["y"][0]
    return out


# revision 23
# speedup vs baseline: 1.0859x; 1.0402x over previous
"""Trainium2 Bass kernel for nn_CholecFixScore (pairwise-IoU mask scoring).

Math (per sample n):
    Gp (P=16, HW) and Gt (T=8, HW) are binary {0,1} masks.
    inters[p,t] = sum_hw Gp[p]*Gt[t];  sp[p] = sum Gp[p];  st[t] = sum Gt[t]
    iou = inters / max(sp+st-inters, 1)            (union==0 => inters==0 => iou 0)
    w[p] = max_t iou[p,t]
    den[hw] = sum_p Gp[p,hw];  r = 1/max(den,1)    (den==0 pixels have Gp==0)
    score[n] = (1/HW) * sum_p w[p] * S[p],  S[p] = sum_hw Gp[p,hw]*r[hw]
which equals the reference's mean over pixels of (sum_p w[p]Gp[p,hw])/den[hw].

Sharding: pure data parallel, 2 samples per core on 8 cores.

Host-side packing (free wrt HW time): masks are {0,1}, EXACT in fp8e4m3,
which halves HBM traffic again vs bf16 and enables DoubleRow matmuls.
Pixel index hw = k*392 + j with k = SBUF partition (128), j in [0,392).
j is chunked as j = c*7 + js  (c in [0,56), js in [0,7)).
  gpw[s] (128, 56*7*17) fp8, free = (c, js, p'):  p' = 16 Gp masks | ones row
  gte[s] (128, 56*77)  fp8, free = (c, x): x = (js,u) u = 8 Gt | ones (63 cols)
                                         then (js, r_hi|r_lo)     (14 cols)
The ones ROW (p'=16) makes the main GEMM emit rhs column sums == st[t];
the ones COLUMN (u=8) emits sp[p].  r is computed on-chip and written as
an exact-ish two-term fp8 split r = r_hi + r_lo (rel err ~2^-8 << 2e-2).

Main GEMM per sample: 28 DoubleRow fp8 matmuls (2 j-chunks each):
  lhsT = gpw view (128, 2, 119), rhs = gte view (128, 2, 77), contiguous
  psum (119, 77) accumulates.  Valid outputs are the 7 diagonal js-blocks;
eye-selector matmuls relocate+sum them into acc (17, 11) =
  [inters|sp|S_hi|S_lo ; st|...].

den'[k,j] = sum_p' gpw (ALL 17, contiguous innermost reduce) = den+1;
den = max(den'-1, 1) fused in one dual-op tensor_scalar; 1/den via
reciprocal_approx_fast.  All DMAs are plain HWDGE with 128 contiguous
descriptors, split over BOTH hwdge queues (sync + scalar) because one
queue issues only ~1.5 DMA/us.  Dummy matmuls reading each arrived gpw
DMA piece keep the PE clock at 2.4 GHz through the DMA phase.
"""

import numpy as np
import ml_dtypes

import concourse.bass as bass
import concourse.tile as tile
from concourse import mybir
from concourse.bass_utils import run_bass_kernel_spmd

F32 = mybir.dt.float32
BF16 = mybir.dt.bfloat16
FP8 = mybir.dt.float8e4
ADD = mybir.AluOpType.add
NPF8 = ml_dtypes.float8_e4m3fn

N, P, T = 16, 16, 8
H, W = 224, 224
HW = H * W            # 50176
PART = 128
JW = HW // PART       # 392 j values per partition
JS = 7                # j values per chunk
NCH = JW // JS        # 56 chunks
NPAIR = NCH // 2      # 28 DoubleRow chunk pairs
PP = P + 1            # 16 masks + ones row
UA = T + 1            # u-part: 8 Gt | ones
MCH = JS * PP         # 119 lhsT cols per chunk
UCOLS = JS * UA       # 63 u-part cols per chunk
RCOLS = JS * 2        # 14 r cols per chunk (js-major, hi|lo)
XCH = UCOLS + RCOLS   # 77 rhs cols per chunk
GPW_COLS = NCH * MCH  # 6664
GTE_COLS = NCH * XCH  # 4312
NCORES = 8
SPC = N // NCORES     # samples per core = 2
INV_HW = 1.0 / HW
USE_DR = False        # DoubleRow perf mode (fp8, 2 k-tiles per matmul)
# DMA pieces: gpw halves, gte thirds (chunk-pair aligned: 20|18|18 chunks)
GTE_CUTS = [0, 20, 38, NCH]
N_WARM_PER_PIECE = 5  # dummy matmuls gated on each gpw DMA piece


def _split_multi_waits(nc):
    """The pinned walrus encodes only ONE sync-wait per instruction; split
    Tile-emitted multi-wait instructions into single-wait NOPs ahead of them
    (same engine, program order => identical semantics)."""
    n = 0
    for f in nc.m.functions:
        for bb in f.blocks:
            insts = bb.instructions
            newlist = []
            changed = False
            for ins in insts:
                si = ins.sync_info
                if si is not None and si.on_wait is not None and len(si.on_wait) > 1:
                    waits = list(si.on_wait)
                    for w in waits[:-1]:
                        n += 1
                        newlist.append(
                            mybir.InstNoOp(
                                name=f"I-waitsplit-{n}",
                                engine=ins.engine,
                                ins=[],
                                outs=[],
                                sync_info=mybir.SyncInfo(on_wait=[w], on_update=[]),
                            )
                        )
                    ins.sync_info = mybir.SyncInfo(
                        on_wait=[waits[-1]], on_update=list(si.on_update or [])
                    )
                    changed = True
                newlist.append(ins)
            if changed:
                while len(insts):
                    insts.pop()
                for x in newlist:
                    insts.append(x)
    return n


def _build():
    nc = bass.Bass("TRN2", target_bir_lowering=False, debug=False)
    gpw = nc.dram_tensor("gpw", [SPC, PART, GPW_COLS], FP8, kind="ExternalInput")
    gte = nc.dram_tensor("gte", [SPC, PART, GTE_COLS], FP8, kind="ExternalInput")
    # ce = [ eye(128) | sel16 | ones16 ]:
    #   sel16[k, m] = 1 iff k == P  (broadcasts acc's st row to partitions 0..15)
    #   ones16[k] = 1 iff k < 16    (final score reduction column)
    ce = nc.dram_tensor("ce", [PART, PART + 17], F32, kind="ExternalInput")
    y = nc.dram_tensor("y", [1, SPC], F32, kind="ExternalOutput")

    with tile.TileContext(nc) as tc:
        with (
            tc.tile_pool(name="big", bufs=2) as big,
            tc.tile_pool(name="scratch", bufs=2) as scratch,
            tc.tile_pool(name="small", bufs=2) as small,
            tc.tile_pool(name="singles", bufs=1) as singles,
            tc.tile_pool(name="psmain", bufs=2, space="PSUM") as psmain,
            tc.tile_pool(name="pswarm", bufs=1, space="PSUM") as pswarm,
            tc.tile_pool(name="psaux", bufs=1, space="PSUM") as psaux,
        ):
            e_sb = singles.tile([PART, PART + 17], F32)
            out_sb = singles.tile([1, SPC], F32)
            lnbias = singles.tile([PART, 1], F32)

            gpws = [big.tile([PART, GPW_COLS], FP8, tag="gpw", name=f"gpw{s}")
                    for s in range(SPC)]
            gtes = [big.tile([PART, GTE_COLS], FP8, tag="gte", name=f"gte{s}")
                    for s in range(SPC)]

            # ---- input DMAs: plain HWDGE, 128 contiguous descriptors each,
            # split across both hwdge queues (sync issues ~650ns/DMA).
            # Arrival order: gpw0, gpw1 (halves in parallel on the 2 queues),
            # then gte pieces interleaved s0/s1 so both main GEMMs chase the
            # tail of the DMA stream. ----
            # gpw0 quartered (2 per queue) so den0 pipelines behind arrival;
            # gpw1/gte halved across the queues.  12 DMAs total; early ones
            # complete before their semaphore lanes recycle.  high_priority
            # hoists the first DMAs ahead of the engines' register init.
            GQ = GPW_COLS // 4
            with tc.high_priority():
                nc.gpsimd.memset(lnbias[:, :], float(2.0 ** -14 - 1.0))
                nc.scalar.dma_start(out=e_sb[:, :], in_=ce[:, :])
                for q in range(4):
                    eng = nc.sync if q % 2 == 0 else nc.scalar
                    eng.dma_start(
                        out=gpws[0][:, q * GQ : (q + 1) * GQ],
                        in_=gpw[0, :, q * GQ : (q + 1) * GQ],
                    )
            GH = GPW_COLS // 2
            GTH = (GTE_CUTS[1]) * XCH  # piece 1 boundary (20 chunks)
            nc.sync.dma_start(out=gpws[1][:, 0:GH], in_=gpw[1, :, 0:GH])
            nc.scalar.dma_start(
                out=gpws[1][:, GH:GPW_COLS], in_=gpw[1, :, GH:GPW_COLS]
            )
            nc.sync.dma_start(out=gtes[0][:, 0:GTH], in_=gte[0, :, 0:GTH])
            nc.scalar.dma_start(
                out=gtes[0][:, GTH:GTE_COLS], in_=gte[0, :, GTH:GTE_COLS]
            )
            nc.sync.dma_start(out=gtes[1][:, 0:GTH], in_=gte[1, :, 0:GTH])
            nc.scalar.dma_start(
                out=gtes[1][:, GTH:GTE_COLS], in_=gte[1, :, GTH:GTE_COLS]
            )

            # ---- PE warmup: HAM releases the clock gate after ~3.4us of
            # sustained activity.  Gate dummy matmuls on the arriving gpw
            # pieces so the array stays busy through the whole DMA phase and
            # the main GEMMs run at 2.4 GHz.  Results are never read. ----
            warm_ps = pswarm.tile([MCH, 512], F32)
            for q in range(4):
                for k in range(3):
                    base = q * GQ
                    nc.tensor.matmul(
                        warm_ps[:, :],
                        gpws[0][:, base : base + MCH],
                        gpws[0][:, base + MCH + k * 512 : base + MCH + (k + 1) * 512],
                    )
            for half in range(2):
                for k in range(2):
                    base = half * GH
                    nc.tensor.matmul(
                        warm_ps[:, :],
                        gpws[1][:, base : base + MCH],
                        gpws[1][:, base + MCH + k * 512 : base + MCH + (k + 1) * 512],
                    )

            # ---- den/r per sample (DVE): den' = sum over ALL 17 p' (incl
            # ones row) = den+1, contiguous innermost reduce, one per gpw
            # half; den = max(den'-1, 1) fused; r = 1/den approx (fp32);
            # two-term fp8 split r = r_hi + r_lo written into gte r slots. ----
            rparts = {}
            denps = {}

            def den_reduce(s, frm, to):
                # DVE: den'[k,(c,js)] = sum over all 17 p' (contiguous
                # innermost) = den+1, for j in [frm, to)
                if s not in denps:
                    denps[s] = scratch.tile(
                        [PART, JW], BF16, tag="denp", name=f"denp{s}"
                    )
                gv = gpws[s][:].rearrange("part (cj p) -> part cj p", p=PP)
                with nc.allow_low_precision(
                    reason="den is an integer <=17; exact in bf16"
                ):
                    nc.vector.tensor_reduce(
                        out=denps[s][:, frm:to],
                        in_=gv[:, frm:to, :],
                        axis=mybir.AxisListType.X,
                        op=ADD,
                    )

            def r_chain(s, h):
                # r = 1/max(den'-1, 1) via exp(-ln(.)) on the ACT engine
                # (table err ~1e-3 << 2e-2; bass bans the Reciprocal table)
                if s not in rparts:
                    rparts[s] = (
                        scratch.tile([PART, JW], F32, tag="lnv", name=f"lnv{s}"),
                        scratch.tile([PART, JW], F32, tag="r32", name=f"r32{s}"),
                        scratch.tile([PART, JW], F32, tag="den32", name=f"den32{s}"),
                    )
                lnv, r32, den32 = rparts[s]
                HJ = JW // 2
                sl = slice(h * HJ, (h + 1) * HJ)
                nc.vector.tensor_scalar(
                    out=den32[:, sl],
                    in0=denps[s][:, sl],
                    scalar1=-1.0,
                    scalar2=1.0,
                    op0=ADD,
                    op1=mybir.AluOpType.max,
                )
                nc.scalar.activation(
                    out=lnv[:, sl], in_=den32[:, sl],
                    func=mybir.ActivationFunctionType.Ln,
                )
                nc.scalar.activation(
                    out=r32[:, sl], in_=lnv[:, sl],
                    func=mybir.ActivationFunctionType.Exp, scale=-1.0,
                )

            # r scatter into the gte r slots (after that piece's DMA landed)
            def scatter_r(s, piece):
                r32 = rparts[s][1]
                clo, chi = GTE_CUTS[piece], GTE_CUTS[piece + 1]
                rv = gtes[s][:].rearrange("part (c x) -> part c x", x=XCH)[
                    :, clo:chi, UCOLS:XCH
                ].rearrange("part c (js two) -> part c js two", two=2)
                rsrc = r32[:].rearrange(
                    "part (c js one) -> part c js one", js=JS, one=1
                )[:, clo:chi, :, :]
                with nc.allow_low_precision(reason="fp8 r split scatter"):
                    nc.gpsimd.tensor_copy(rv[:, :, :, 0:1], rsrc)
                    nc.gpsimd.tensor_tensor(
                        rv[:, :, :, 1:2], rsrc, rv[:, :, :, 0:1],
                        mybir.AluOpType.subtract,
                    )

            # ---- main GEMM: DoubleRow fp8, 2 j-chunks per matmul, issued
            # piece-by-piece chasing the gte DMA stream ----
            pss = {
                s: psmain.tile([MCH, XCH], F32, tag="main", name=f"ps_main{s}")
                for s in range(SPC)
            }

            def main_piece(s, piece):
                lo2, hi2 = GTE_CUTS[piece] // 2, GTE_CUTS[piece + 1] // 2
                gpv = gpws[s][:].rearrange(
                    "part (c2 two m) -> part c2 two m", two=2, m=MCH
                )
                gtv = gtes[s][:].rearrange(
                    "part (c2 two x) -> part c2 two x", two=2, x=XCH
                )
                for c2 in range(lo2, hi2):
                    if USE_DR:
                        nc.tensor.matmul(
                            pss[s][:, :],
                            gpv[:, c2],
                            gtv[:, c2],
                            start=(c2 == 0),
                            stop=(c2 == NPAIR - 1),
                            perf_mode=mybir.MatmulPerfMode.DoubleRow,
                        )
                    else:
                        for c in (2 * c2, 2 * c2 + 1):
                            nc.tensor.matmul(
                                pss[s][:, :],
                                gpws[s][:, c * MCH : (c + 1) * MCH],
                                gtes[s][:, c * XCH : (c + 1) * XCH],
                                start=(c == 0),
                                stop=(c == NCH - 1),
                            )

            accs = {}

            def extract(s):
                ext = small.tile([MCH, XCH], F32, tag="ext", name=f"ext{s}")
                nc.vector.tensor_copy(ext[:, :], pss[s][:, :])
                ps_acc = psaux.tile([PP, UA + 2], F32, tag="acc", name=f"ps_acc{s}")
                for js in range(JS):
                    nc.tensor.matmul(
                        ps_acc[:, 0:UA],
                        e_sb[0:MCH, js * PP : (js + 1) * PP],
                        ext[:, js * UA : (js + 1) * UA],
                        start=(js == 0),
                        stop=(js == JS - 1),
                    )
                for js in range(JS):
                    nc.tensor.matmul(
                        ps_acc[:, UA : UA + 2],
                        e_sb[0:MCH, js * PP : (js + 1) * PP],
                        ext[:, UCOLS + js * 2 : UCOLS + js * 2 + 2],
                        start=(js == 0),
                        stop=(js == JS - 1),
                    )
                acc = small.tile([PP, UA + 2], F32, tag="accsb", name=f"acc{s}")
                nc.vector.tensor_copy(acc[:, :], ps_acc[:, :])
                accs[s] = acc

            def finish(s):
                acc = accs[s]
                # broadcast st (ones row of acc) to 16 partitions via sel16
                ps_st16 = psaux.tile([16, T], F32, tag="st16", name=f"ps_st16{s}")
                nc.tensor.matmul(
                    ps_st16[:, :], e_sb[0:PP, PART : PART + 16], acc[0:PP, 0:T]
                )
                # unions = max((st16 + sp) - inters, 1);  iou = inters/unions
                unions = small.tile([16, T], F32, tag=f"un{s}", name=f"unions{s}")
                nc.vector.scalar_tensor_tensor(
                    out=unions[:, :],
                    in0=ps_st16[:, :],
                    scalar=acc[0:P, T : T + 1],
                    in1=acc[0:P, 0:T],
                    op0=ADD,
                    op1=mybir.AluOpType.subtract,
                )
                nc.vector.tensor_scalar_max(
                    out=unions[:, :], in0=unions[:, :], scalar1=1.0
                )
                uln = small.tile([16, T], F32, tag=f"ul{s}", name=f"uln{s}")
                nc.scalar.activation(
                    out=uln[:, :], in_=unions[:, :],
                    func=mybir.ActivationFunctionType.Ln,
                )
                uinv = small.tile([16, T], F32, tag=f"ui{s}", name=f"uinv{s}")
                nc.scalar.activation(
                    out=uinv[:, :], in_=uln[:, :],
                    func=mybir.ActivationFunctionType.Exp, scale=-1.0,
                )
                iou = small.tile([16, T], F32, tag=f"iou{s}", name=f"iou{s}")
                nc.vector.tensor_tensor(
                    iou[:, :], acc[0:P, 0:T], uinv[:, :], mybir.AluOpType.mult
                )
                wmax = small.tile([16, 1], F32, tag=f"wm{s}", name=f"wmax{s}")
                nc.vector.tensor_reduce(
                    out=wmax[:, :],
                    in_=iou[:, :],
                    axis=mybir.AxisListType.X,
                    op=mybir.AluOpType.max,
                )
                # ws = (S_hi + S_lo) * w
                ws = small.tile([16, 1], F32, tag=f"ws{s}", name=f"ws{s}")
                nc.vector.scalar_tensor_tensor(
                    out=ws[:, :],
                    in0=acc[0:P, UA : UA + 1],
                    scalar=acc[0:P, UA + 1 : UA + 2],
                    in1=wmax[:, :],
                    op0=ADD,
                    op1=mybir.AluOpType.mult,
                )
                ps_score = psaux.tile([1, 1], F32, tag="sc", name=f"ps_score{s}")
                nc.tensor.matmul(
                    ps_score[:, :], e_sb[0:16, PART + 16 : PART + 17], ws[:, :]
                )
                nc.vector.tensor_scalar_mul(
                    out=out_sb[0:1, s : s + 1], in0=ps_score[:, :], scalar1=INV_HW
                )

            QJ = JW // 4
            HJ = JW // 2
            # s0 half1 (quarters), chain, scatter piece0 -> main0 p0 early
            den_reduce(0, 0, QJ)
            den_reduce(0, QJ, 2 * QJ)
            r_chain(0, 0)
            scatter_r(0, 0)
            main_piece(0, 0)
            # s1 half1 -> main1 p0
            den_reduce(1, 0, HJ)
            r_chain(1, 0)
            scatter_r(1, 0)
            main_piece(1, 0)
            # s0 half2 -> main0 p1, p2
            den_reduce(0, 2 * QJ, 3 * QJ)
            den_reduce(0, 3 * QJ, JW)
            r_chain(0, 1)
            scatter_r(0, 1)
            scatter_r(0, 2)
            main_piece(0, 1)
            main_piece(0, 2)
            extract(0)
            # s1 half2 -> main1 p1, p2
            den_reduce(1, HJ, JW)
            r_chain(1, 1)
            scatter_r(1, 1)
            scatter_r(1, 2)
            main_piece(1, 1)
            main_piece(1, 2)
            extract(1)
            finish(0)
            finish(1)

            nc.sync.dma_start(out=y[:, :], in_=out_sb[:, :])

    _split_multi_waits(nc)
    return nc


_NC = None


def _get_nc():
    global _NC
    if _NC is None:
        _NC = _build()
    return _NC


def _pack(groups_pred: np.ndarray, groups_true: np.ndarray):
    # binarize (match torch .bool(): nonzero -> 1); {0,1} exact in fp8e4m3
    gp = (groups_pred != 0).astype(NPF8)     # (N, P, H, W)
    gt = (groups_true != 0).astype(NPF8)     # (N, T, H, W)
    # (N, P, PART, NCH, JS) -> (N, PART, NCH, JS, P) + ones row
    gp5 = gp.reshape(N, P, PART, NCH, JS).transpose(0, 2, 3, 4, 1)
    gpw = np.empty((N, PART, NCH, JS, PP), dtype=NPF8)
    gpw[..., 0:P] = gp5
    gpw[..., P] = NPF8(1.0)
    gt5 = gt.reshape(N, T, PART, NCH, JS).transpose(0, 2, 3, 4, 1)
    gte = np.zeros((N, PART, NCH, XCH), dtype=NPF8)
    upart = np.empty((N, PART, NCH, JS, UA), dtype=NPF8)
    upart[..., 0:T] = gt5
    upart[..., T] = NPF8(1.0)
    gte[..., 0:UCOLS] = upart.reshape(N, PART, NCH, UCOLS)
    gpw = np.ascontiguousarray(gpw.reshape(NCORES, SPC, PART, GPW_COLS))
    gte = np.ascontiguousarray(gte.reshape(NCORES, SPC, PART, GTE_COLS))
    return gpw, gte


def make_in_maps(groups_pred: np.ndarray, groups_true: np.ndarray) -> list[dict]:
    gpw, gte = _pack(groups_pred, groups_true)
    ce = np.zeros((PART, PART + 17), dtype=np.float32)
    ce[:, 0:PART] = np.eye(PART, dtype=np.float32)
    ce[P, PART : PART + 16] = 1.0
    ce[0:16, PART + 16] = 1.0
    return [{"gpw": gpw[c], "gte": gte[c], "ce": ce} for c in range(NCORES)]


def kernel(groups_pred: np.ndarray, groups_true: np.ndarray) -> np.ndarray:
    assert groups_pred.shape == (N, P, H, W)
    assert groups_true.shape == (N, T, H, W)
    in_maps = make_in_maps(groups_pred, groups_true)
    res = run_bass_kernel_spmd(_get_nc(), in_maps, core_ids=list(range(NCORES)))
    out = np.empty((N,), dtype=np.float32)
    for c in range(NCORES):
        out[c * SPC : (c + 1) * SPC] = res.results[c]["y"][0]
    return out


# revision 24
# speedup vs baseline: 1.1425x; 1.0522x over previous
"""Trainium2 Bass kernel for nn_CholecFixScore (pairwise-IoU mask scoring).

Math (per sample n):
    Gp (P=16, HW) and Gt (T=8, HW) are binary {0,1} masks.
    inters[p,t] = sum_hw Gp[p]*Gt[t];  sp[p] = sum Gp[p];  st[t] = sum Gt[t]
    iou = inters / max(sp+st-inters, 1)            (union==0 => inters==0 => iou 0)
    w[p] = max_t iou[p,t]
    den[hw] = sum_p Gp[p,hw];  r = 1/max(den,1)    (den==0 pixels have Gp==0)
    score[n] = (1/HW) * sum_p w[p] * S[p],  S[p] = sum_hw Gp[p,hw]*r[hw]
which equals the reference's mean over pixels of (sum_p w[p]Gp[p,hw])/den[hw].

Sharding: pure data parallel, 2 samples per core on 8 cores.

Host-side packing (free wrt HW time): masks are {0,1}, EXACT in fp8e4m3,
which halves HBM traffic again vs bf16 and enables DoubleRow matmuls.
Pixel index hw = k*392 + j with k = SBUF partition (128), j in [0,392).
j is chunked as j = c*7 + js  (c in [0,56), js in [0,7)).
  gpw[s] (128, 56*7*17) fp8, free = (c, js, p'):  p' = 16 Gp masks | ones row
  gte[s] (128, 56*77)  fp8, free = (c, x): x = (js,u) u = 8 Gt | ones (63 cols)
                                         then (js, r_hi|r_lo)     (14 cols)
The ones ROW (p'=16) makes the main GEMM emit rhs column sums == st[t];
the ones COLUMN (u=8) emits sp[p].  r is computed on-chip and written as
an exact-ish two-term fp8 split r = r_hi + r_lo (rel err ~2^-8 << 2e-2).

Main GEMM per sample: 28 DoubleRow fp8 matmuls (2 j-chunks each):
  lhsT = gpw view (128, 2, 119), rhs = gte view (128, 2, 77), contiguous
  psum (119, 77) accumulates.  Valid outputs are the 7 diagonal js-blocks;
eye-selector matmuls relocate+sum them into acc (17, 11) =
  [inters|sp|S_hi|S_lo ; st|...].

den'[k,j] = sum_p' gpw (ALL 17, contiguous innermost reduce) = den+1;
den = max(den'-1, 1) fused in one dual-op tensor_scalar; 1/den via
reciprocal_approx_fast.  All DMAs are plain HWDGE with 128 contiguous
descriptors, split over BOTH hwdge queues (sync + scalar) because one
queue issues only ~1.5 DMA/us.  Dummy matmuls reading each arrived gpw
DMA piece keep the PE clock at 2.4 GHz through the DMA phase.
"""

import numpy as np
import ml_dtypes

import concourse.bass as bass
import concourse.tile as tile
from concourse import mybir
from concourse.bass_utils import run_bass_kernel_spmd

F32 = mybir.dt.float32
BF16 = mybir.dt.bfloat16
FP8 = mybir.dt.float8e4
ADD = mybir.AluOpType.add
NPF8 = ml_dtypes.float8_e4m3fn

N, P, T = 16, 16, 8
H, W = 224, 224
HW = H * W            # 50176
PART = 128
JW = HW // PART       # 392 j values per partition
JS = 7                # j values per chunk
NCH = JW // JS        # 56 chunks
NPAIR = NCH // 2      # 28 DoubleRow chunk pairs
PP = P + 1            # 16 masks + ones row
UA = T + 1            # u-part: 8 Gt | ones
MCH = JS * PP         # 119 lhsT cols per chunk
UCOLS = JS * UA       # 63 u-part cols per chunk
RCOLS = JS * 2        # 14 r cols per chunk (js-major, hi|lo)
XCH = UCOLS + RCOLS   # 77 rhs cols per chunk
GPW_COLS = NCH * MCH  # 6664
GTE_COLS = NCH * XCH  # 4312
NCORES = 8
SPC = N // NCORES     # samples per core = 2
INV_HW = 1.0 / HW
USE_DR = False        # DoubleRow perf mode (fp8, 2 k-tiles per matmul)
# DMA pieces: gpw halves, gte thirds (chunk-pair aligned: 20|18|18 chunks)
GTE_CUTS = [0, 20, 38, NCH]
N_WARM_PER_PIECE = 5  # dummy matmuls gated on each gpw DMA piece


def _split_multi_waits(nc):
    """The pinned walrus encodes only ONE sync-wait per instruction; split
    Tile-emitted multi-wait instructions into single-wait NOPs ahead of them
    (same engine, program order => identical semantics)."""
    n = 0
    for f in nc.m.functions:
        for bb in f.blocks:
            insts = bb.instructions
            newlist = []
            changed = False
            for ins in insts:
                si = ins.sync_info
                if si is not None and si.on_wait is not None and len(si.on_wait) > 1:
                    waits = list(si.on_wait)
                    for w in waits[:-1]:
                        n += 1
                        newlist.append(
                            mybir.InstNoOp(
                                name=f"I-waitsplit-{n}",
                                engine=ins.engine,
                                ins=[],
                                outs=[],
                                sync_info=mybir.SyncInfo(on_wait=[w], on_update=[]),
                            )
                        )
                    ins.sync_info = mybir.SyncInfo(
                        on_wait=[waits[-1]], on_update=list(si.on_update or [])
                    )
                    changed = True
                newlist.append(ins)
            if changed:
                while len(insts):
                    insts.pop()
                for x in newlist:
                    insts.append(x)
    return n


def _build():
    nc = bass.Bass("TRN2", target_bir_lowering=False, debug=False)
    gpw = nc.dram_tensor("gpw", [SPC, PART, GPW_COLS], FP8, kind="ExternalInput")
    gte = nc.dram_tensor("gte", [SPC, PART, GTE_COLS], FP8, kind="ExternalInput")
    # ce = [ eye(128) | sel16 | ones16 ]:
    #   sel16[k, m] = 1 iff k == P  (broadcasts acc's st row to partitions 0..15)
    #   ones16[k] = 1 iff k < 16    (final score reduction column)
    ce = nc.dram_tensor("ce", [PART, PART + 17], F32, kind="ExternalInput")
    y = nc.dram_tensor("y", [1, SPC], F32, kind="ExternalOutput")

    with tile.TileContext(nc) as tc:
        with (
            tc.tile_pool(name="big", bufs=2) as big,
            tc.tile_pool(name="scratch", bufs=2) as scratch,
            tc.tile_pool(name="small", bufs=2) as small,
            tc.tile_pool(name="singles", bufs=1) as singles,
            tc.tile_pool(name="psmain", bufs=2, space="PSUM") as psmain,
            tc.tile_pool(name="pswarm", bufs=1, space="PSUM") as pswarm,
            tc.tile_pool(name="psaux", bufs=1, space="PSUM") as psaux,
        ):
            e_sb = singles.tile([PART, PART + 17], F32)
            out_sb = singles.tile([1, SPC], F32)
            lnbias = singles.tile([PART, 1], F32)

            gpws = [big.tile([PART, GPW_COLS], FP8, tag="gpw", name=f"gpw{s}")
                    for s in range(SPC)]
            gtes = [big.tile([PART, GTE_COLS], FP8, tag="gte", name=f"gte{s}")
                    for s in range(SPC)]

            # ---- input DMAs: plain HWDGE, 128 contiguous descriptors each,
            # split across both hwdge queues (sync issues ~650ns/DMA).
            # Arrival order: gpw0, gpw1 (halves in parallel on the 2 queues),
            # then gte pieces interleaved s0/s1 so both main GEMMs chase the
            # tail of the DMA stream. ----
            # gpw0 quartered (2 per queue) so den0 pipelines behind arrival;
            # gpw1/gte halved across the queues.  12 DMAs total; early ones
            # complete before their semaphore lanes recycle.  high_priority
            # hoists the first DMAs ahead of the engines' register init.
            GQ = GPW_COLS // 4
            with tc.high_priority():
                nc.gpsimd.memset(lnbias[:, :], -2.0)
                nc.scalar.dma_start(out=e_sb[:, :], in_=ce[:, :])
                for q in range(4):
                    eng = nc.sync if q % 2 == 0 else nc.scalar
                    eng.dma_start(
                        out=gpws[0][:, q * GQ : (q + 1) * GQ],
                        in_=gpw[0, :, q * GQ : (q + 1) * GQ],
                    )
            GH = GPW_COLS // 2
            GTH = (GTE_CUTS[1]) * XCH  # piece 1 boundary (20 chunks)
            nc.sync.dma_start(out=gpws[1][:, 0:GH], in_=gpw[1, :, 0:GH])
            nc.scalar.dma_start(
                out=gpws[1][:, GH:GPW_COLS], in_=gpw[1, :, GH:GPW_COLS]
            )
            nc.sync.dma_start(out=gtes[0][:, 0:GTH], in_=gte[0, :, 0:GTH])
            nc.scalar.dma_start(
                out=gtes[0][:, GTH:GTE_COLS], in_=gte[0, :, GTH:GTE_COLS]
            )
            nc.sync.dma_start(out=gtes[1][:, 0:GTH], in_=gte[1, :, 0:GTH])
            nc.scalar.dma_start(
                out=gtes[1][:, GTH:GTE_COLS], in_=gte[1, :, GTH:GTE_COLS]
            )

            # ---- PE warmup: HAM releases the clock gate after ~3.4us of
            # sustained activity.  Gate dummy matmuls on the arriving gpw
            # pieces so the array stays busy through the whole DMA phase and
            # the main GEMMs run at 2.4 GHz.  Results are never read. ----
            warm_ps = pswarm.tile([MCH, 512], F32)
            for q in range(4):
                for k in range(3):
                    base = q * GQ
                    nc.tensor.matmul(
                        warm_ps[:, :],
                        gpws[0][:, base : base + MCH],
                        gpws[0][:, base + MCH + k * 512 : base + MCH + (k + 1) * 512],
                    )
            for half in range(2):
                for k in range(2):
                    base = half * GH
                    nc.tensor.matmul(
                        warm_ps[:, :],
                        gpws[1][:, base : base + MCH],
                        gpws[1][:, base + MCH + k * 512 : base + MCH + (k + 1) * 512],
                    )

            # ---- den/r per sample (DVE): den' = sum over ALL 17 p' (incl
            # ones row) = den+1, contiguous innermost reduce, one per gpw
            # half; den = max(den'-1, 1) fused; r = 1/den approx (fp32);
            # two-term fp8 split r = r_hi + r_lo written into gte r slots. ----
            rparts = {}
            denps = {}

            def den_reduce(s, frm, to):
                # DVE: den'[k,(c,js)] = sum over all 17 p' (contiguous
                # innermost) = den+1, for j in [frm, to)
                if s not in denps:
                    denps[s] = scratch.tile(
                        [PART, JW], BF16, tag="denp", name=f"denp{s}"
                    )
                gv = gpws[s][:].rearrange("part (cj p) -> part cj p", p=PP)
                with nc.allow_low_precision(
                    reason="den is an integer <=17; exact in bf16"
                ):
                    nc.vector.tensor_reduce(
                        out=denps[s][:, frm:to],
                        in_=gv[:, frm:to, :],
                        axis=mybir.AxisListType.X,
                        op=ADD,
                    )

            def r_chain(s, h):
                # r = 1/max(den'-1, 1) via exp(-ln(.)) on the ACT engine
                # (table err ~1e-3 << 2e-2; bass bans the Reciprocal table)
                if s not in rparts:
                    rparts[s] = (
                        scratch.tile([PART, JW], F32, tag="lnv", name=f"lnv{s}"),
                        scratch.tile([PART, JW], F32, tag="r32", name=f"r32{s}"),
                        scratch.tile([PART, JW], F32, tag="den32", name=f"den32{s}"),
                    )
                lnv, r32, den32 = rparts[s]
                HJ = JW // 2
                sl = slice(h * HJ, (h + 1) * HJ)
                # max(den'-1, 1) = Relu(den'-2) + 1, all on the ACT engine
                nc.scalar.activation(
                    out=den32[:, sl], in_=denps[s][:, sl],
                    func=mybir.ActivationFunctionType.Relu,
                    bias=lnbias[:, :],
                )
                nc.scalar.activation(
                    out=lnv[:, sl], in_=den32[:, sl],
                    func=mybir.ActivationFunctionType.Ln,
                    bias=1.0,
                )
                nc.scalar.activation(
                    out=r32[:, sl], in_=lnv[:, sl],
                    func=mybir.ActivationFunctionType.Exp, scale=-1.0,
                )

            # r scatter into the gte r slots (after that piece's DMA landed)
            def scatter_r(s, piece):
                r32 = rparts[s][1]
                clo, chi = GTE_CUTS[piece], GTE_CUTS[piece + 1]
                rv = gtes[s][:].rearrange("part (c x) -> part c x", x=XCH)[
                    :, clo:chi, UCOLS:XCH
                ].rearrange("part c (js two) -> part c js two", two=2)
                rsrc = r32[:].rearrange(
                    "part (c js one) -> part c js one", js=JS, one=1
                )[:, clo:chi, :, :]
                with nc.allow_low_precision(reason="fp8 r split scatter"):
                    nc.gpsimd.tensor_copy(rv[:, :, :, 0:1], rsrc)
                    nc.gpsimd.tensor_tensor(
                        rv[:, :, :, 1:2], rsrc, rv[:, :, :, 0:1],
                        mybir.AluOpType.subtract,
                    )

            # ---- main GEMM: DoubleRow fp8, 2 j-chunks per matmul, issued
            # piece-by-piece chasing the gte DMA stream ----
            pss = {
                s: psmain.tile([MCH, XCH], F32, tag="main", name=f"ps_main{s}")
                for s in range(SPC)
            }

            def main_piece(s, piece):
                lo2, hi2 = GTE_CUTS[piece] // 2, GTE_CUTS[piece + 1] // 2
                gpv = gpws[s][:].rearrange(
                    "part (c2 two m) -> part c2 two m", two=2, m=MCH
                )
                gtv = gtes[s][:].rearrange(
                    "part (c2 two x) -> part c2 two x", two=2, x=XCH
                )
                for c2 in range(lo2, hi2):
                    if USE_DR:
                        nc.tensor.matmul(
                            pss[s][:, :],
                            gpv[:, c2],
                            gtv[:, c2],
                            start=(c2 == 0),
                            stop=(c2 == NPAIR - 1),
                            perf_mode=mybir.MatmulPerfMode.DoubleRow,
                        )
                    else:
                        for c in (2 * c2, 2 * c2 + 1):
                            nc.tensor.matmul(
                                pss[s][:, :],
                                gpws[s][:, c * MCH : (c + 1) * MCH],
                                gtes[s][:, c * XCH : (c + 1) * XCH],
                                start=(c == 0),
                                stop=(c == NCH - 1),
                            )

            accs = {}

            def extract(s):
                ext = small.tile([MCH, XCH], F32, tag="ext", name=f"ext{s}")
                nc.vector.tensor_copy(ext[:, :], pss[s][:, :])
                ps_acc = psaux.tile([PP, UA + 2], F32, tag="acc", name=f"ps_acc{s}")
                for js in range(JS):
                    nc.tensor.matmul(
                        ps_acc[:, 0:UA],
                        e_sb[0:MCH, js * PP : (js + 1) * PP],
                        ext[:, js * UA : (js + 1) * UA],
                        start=(js == 0),
                        stop=(js == JS - 1),
                    )
                for js in range(JS):
                    nc.tensor.matmul(
                        ps_acc[:, UA : UA + 2],
                        e_sb[0:MCH, js * PP : (js + 1) * PP],
                        ext[:, UCOLS + js * 2 : UCOLS + js * 2 + 2],
                        start=(js == 0),
                        stop=(js == JS - 1),
                    )
                acc = small.tile([PP, UA + 2], F32, tag="accsb", name=f"acc{s}")
                nc.vector.tensor_copy(acc[:, :], ps_acc[:, :])
                accs[s] = acc

            def finish(s):
                acc = accs[s]
                # broadcast st (ones row of acc) to 16 partitions via sel16
                ps_st16 = psaux.tile([16, T], F32, tag="st16", name=f"ps_st16{s}")
                nc.tensor.matmul(
                    ps_st16[:, :], e_sb[0:PP, PART : PART + 16], acc[0:PP, 0:T]
                )
                # unions = max((st16 + sp) - inters, 1);  iou = inters/unions
                unions = small.tile([16, T], F32, tag=f"un{s}", name=f"unions{s}")
                nc.vector.scalar_tensor_tensor(
                    out=unions[:, :],
                    in0=ps_st16[:, :],
                    scalar=acc[0:P, T : T + 1],
                    in1=acc[0:P, 0:T],
                    op0=ADD,
                    op1=mybir.AluOpType.subtract,
                )
                nc.vector.tensor_scalar_max(
                    out=unions[:, :], in0=unions[:, :], scalar1=1.0
                )
                uln = small.tile([16, T], F32, tag=f"ul{s}", name=f"uln{s}")
                nc.scalar.activation(
                    out=uln[:, :], in_=unions[:, :],
                    func=mybir.ActivationFunctionType.Ln,
                )
                uinv = small.tile([16, T], F32, tag=f"ui{s}", name=f"uinv{s}")
                nc.scalar.activation(
                    out=uinv[:, :], in_=uln[:, :],
                    func=mybir.ActivationFunctionType.Exp, scale=-1.0,
                )
                iou = small.tile([16, T], F32, tag=f"iou{s}", name=f"iou{s}")
                nc.vector.tensor_tensor(
                    iou[:, :], acc[0:P, 0:T], uinv[:, :], mybir.AluOpType.mult
                )
                wmax = small.tile([16, 1], F32, tag=f"wm{s}", name=f"wmax{s}")
                nc.vector.tensor_reduce(
                    out=wmax[:, :],
                    in_=iou[:, :],
                    axis=mybir.AxisListType.X,
                    op=mybir.AluOpType.max,
                )
                # ws = (S_hi + S_lo) * w
                ws = small.tile([16, 1], F32, tag=f"ws{s}", name=f"ws{s}")
                nc.vector.scalar_tensor_tensor(
                    out=ws[:, :],
                    in0=acc[0:P, UA : UA + 1],
                    scalar=acc[0:P, UA + 1 : UA + 2],
                    in1=wmax[:, :],
                    op0=ADD,
                    op1=mybir.AluOpType.mult,
                )
                ps_score = psaux.tile([1, 1], F32, tag="sc", name=f"ps_score{s}")
                nc.tensor.matmul(
                    ps_score[:, :], e_sb[0:16, PART + 16 : PART + 17], ws[:, :]
                )
                nc.vector.tensor_scalar_mul(
                    out=out_sb[0:1, s : s + 1], in0=ps_score[:, :], scalar1=INV_HW
                )

            QJ = JW // 4
            HJ = JW // 2
            # s0 half1 (quarters), chain, scatter piece0 -> main0 p0 early
            den_reduce(0, 0, QJ)
            den_reduce(0, QJ, 2 * QJ)
            r_chain(0, 0)
            scatter_r(0, 0)
            main_piece(0, 0)
            # s1 half1 -> main1 p0
            den_reduce(1, 0, HJ)
            r_chain(1, 0)
            scatter_r(1, 0)
            main_piece(1, 0)
            # s0 half2 -> main0 p1, p2
            den_reduce(0, 2 * QJ, 3 * QJ)
            den_reduce(0, 3 * QJ, JW)
            r_chain(0, 1)
            scatter_r(0, 1)
            scatter_r(0, 2)
            main_piece(0, 1)
            main_piece(0, 2)
            extract(0)
            # s1 half2 -> main1 p1, p2
            den_reduce(1, HJ, JW)
            r_chain(1, 1)
            scatter_r(1, 1)
            scatter_r(1, 2)
            main_piece(1, 1)
            main_piece(1, 2)
            extract(1)
            finish(0)
            finish(1)

            nc.sync.dma_start(out=y[:, :], in_=out_sb[:, :])

    _split_multi_waits(nc)
    return nc


_NC = None


def _get_nc():
    global _NC
    if _NC is None:
        _NC = _build()
    return _NC


def _pack(groups_pred: np.ndarray, groups_true: np.ndarray):
    # binarize (match torch .bool(): nonzero -> 1); {0,1} exact in fp8e4m3
    gp = (groups_pred != 0).astype(NPF8)     # (N, P, H, W)
    gt = (groups_true != 0).astype(NPF8)     # (N, T, H, W)
    # (N, P, PART, NCH, JS) -> (N, PART, NCH, JS, P) + ones row
    gp5 = gp.reshape(N, P, PART, NCH, JS).transpose(0, 2, 3, 4, 1)
    gpw = np.empty((N, PART, NCH, JS, PP), dtype=NPF8)
    gpw[..., 0:P] = gp5
    gpw[..., P] = NPF8(1.0)
    gt5 = gt.reshape(N, T, PART, NCH, JS).transpose(0, 2, 3, 4, 1)
    gte = np.zeros((N, PART, NCH, XCH), dtype=NPF8)
    upart = np.empty((N, PART, NCH, JS, UA), dtype=NPF8)
    upart[..., 0:T] = gt5
    upart[..., T] = NPF8(1.0)
    gte[..., 0:UCOLS] = upart.reshape(N, PART, NCH, UCOLS)
    gpw = np.ascontiguousarray(gpw.reshape(NCORES, SPC, PART, GPW_COLS))
    gte = np.ascontiguousarray(gte.reshape(NCORES, SPC, PART, GTE_COLS))
    return gpw, gte


def make_in_maps(groups_pred: np.ndarray, groups_true: np.ndarray) -> list[dict]:
    gpw, gte = _pack(groups_pred, groups_true)
    ce = np.zeros((PART, PART + 17), dtype=np.float32)
    ce[:, 0:PART] = np.eye(PART, dtype=np.float32)
    ce[P, PART : PART + 16] = 1.0
    ce[0:16, PART + 16] = 1.0
    return [{"gpw": gpw[c], "gte": gte[c], "ce": ce} for c in range(NCORES)]


def kernel(groups_pred: np.ndarray, groups_true: np.ndarray) -> np.ndarray:
    assert groups_pred.shape == (N, P, H, W)
    assert groups_true.shape == (N, T, H, W)
    in_maps = make_in_maps(groups_pred, groups_true)
    res = run_bass_kernel_spmd(_get_nc(), in_maps, core_ids=list(range(NCORES)))
    out = np.empty((N,), dtype=np.float32)
    for c in range(NCORES):
        out[c * SPC : (c + 1) * SPC] = res.results[c]["y"][0]
    return out


# revision 26
# speedup vs baseline: 1.1491x; 1.0058x over previous
"""Trainium2 Bass kernel for nn_CholecFixScore (pairwise-IoU mask scoring).

Math (per sample n):
    Gp (P=16, HW) and Gt (T=8, HW) are binary {0,1} masks.
    inters[p,t] = sum_hw Gp[p]*Gt[t];  sp[p] = sum Gp[p];  st[t] = sum Gt[t]
    iou = inters / max(sp+st-inters, 1)            (union==0 => inters==0 => iou 0)
    w[p] = max_t iou[p,t]
    den[hw] = sum_p Gp[p,hw];  r = 1/max(den,1)    (den==0 pixels have Gp==0)
    score[n] = (1/HW) * sum_p w[p] * S[p],  S[p] = sum_hw Gp[p,hw]*r[hw]
which equals the reference's mean over pixels of (sum_p w[p]Gp[p,hw])/den[hw].

Sharding: pure data parallel, 2 samples per core on 8 cores.

Host-side packing (free wrt HW time): masks are {0,1}, EXACT in fp8e4m3,
which halves HBM traffic again vs bf16 and enables DoubleRow matmuls.
Pixel index hw = k*392 + j with k = SBUF partition (128), j in [0,392).
j is chunked as j = c*7 + js  (c in [0,56), js in [0,7)).
  gpw[s] (128, 56*7*17) fp8, free = (c, js, p'):  p' = 16 Gp masks | ones row
  gte[s] (128, 56*77)  fp8, free = (c, x): x = (js,u) u = 8 Gt | ones (63 cols)
                                         then (js, r_hi|r_lo)     (14 cols)
The ones ROW (p'=16) makes the main GEMM emit rhs column sums == st[t];
the ones COLUMN (u=8) emits sp[p].  r is computed on-chip and written as
an exact-ish two-term fp8 split r = r_hi + r_lo (rel err ~2^-8 << 2e-2).

Main GEMM per sample: 56 accumulating fp8 matmuls with fully contiguous
lhsT (119 cols) / rhs (77 cols) APs into psum (119, 77).  Valid outputs
are the 7 diagonal js-blocks; eye-selector matmuls relocate+sum them
into acc (17, 11) = [inters|sp|S_hi|S_lo ; st|...].  (DoubleRow fails
this walrus's ISA check for every shape, so plain fp8 matmuls.)

den'[k,j] = sum_p' gpw (all 17 p', contiguous innermost DVE reduce, one
per gpw DMA piece) = den+1; the whole r chain runs on the ACT engine:
max(den'-1,1) = Relu(den'-2)+1, then 1/den = exp(-ln(den)) (table err
~1e-3; bass bans the Reciprocal table).  The fp8 two-term r split is
scattered straight into the gte r slots by gpsimd (copy + subtract).
All DMAs are plain HWDGE with 128 contiguous descriptors, split over
BOTH hwdge queues (sync + scalar: one queue issues only ~1.5 DMA/us),
ordered gpw0 (quartered), gpw1, gte0, gte1 so den/r for each sample
completes while later DMAs stream, and each main GEMM chases its gte
pieces.  Dummy matmuls reading each arrived gpw piece keep the PE
clock at 2.4 GHz through the DMA phase.
"""

import numpy as np
import ml_dtypes

import concourse.bass as bass
import concourse.tile as tile
from concourse import mybir
from concourse.bass_utils import run_bass_kernel_spmd

F32 = mybir.dt.float32
BF16 = mybir.dt.bfloat16
FP8 = mybir.dt.float8e4
ADD = mybir.AluOpType.add
NPF8 = ml_dtypes.float8_e4m3fn

N, P, T = 16, 16, 8
H, W = 224, 224
HW = H * W            # 50176
PART = 128
JW = HW // PART       # 392 j values per partition
JS = 7                # j values per chunk
NCH = JW // JS        # 56 chunks
NPAIR = NCH // 2      # 28 DoubleRow chunk pairs
PP = P + 1            # 16 masks + ones row
UA = T + 1            # u-part: 8 Gt | ones
MCH = JS * PP         # 119 lhsT cols per chunk
UCOLS = JS * UA       # 63 u-part cols per chunk
RCOLS = JS * 2        # 14 r cols per chunk (js-major, hi|lo)
XCH = UCOLS + RCOLS   # 77 rhs cols per chunk
GPW_COLS = NCH * MCH  # 6664
GTE_COLS = NCH * XCH  # 4312
NCORES = 8
SPC = N // NCORES     # samples per core = 2
INV_HW = 1.0 / HW
USE_DR = False        # DoubleRow perf mode (fp8, 2 k-tiles per matmul)
# DMA pieces: gpw halves, gte thirds (chunk-pair aligned: 20|18|18 chunks)
GTE_CUTS = [0, 20, 38, NCH]
N_WARM_PER_PIECE = 5  # dummy matmuls gated on each gpw DMA piece


def _split_multi_waits(nc):
    """The pinned walrus encodes only ONE sync-wait per instruction; split
    Tile-emitted multi-wait instructions into single-wait NOPs ahead of them
    (same engine, program order => identical semantics)."""
    n = 0
    for f in nc.m.functions:
        for bb in f.blocks:
            insts = bb.instructions
            newlist = []
            changed = False
            for ins in insts:
                si = ins.sync_info
                if si is not None and si.on_wait is not None and len(si.on_wait) > 1:
                    waits = list(si.on_wait)
                    for w in waits[:-1]:
                        n += 1
                        newlist.append(
                            mybir.InstNoOp(
                                name=f"I-waitsplit-{n}",
                                engine=ins.engine,
                                ins=[],
                                outs=[],
                                sync_info=mybir.SyncInfo(on_wait=[w], on_update=[]),
                            )
                        )
                    ins.sync_info = mybir.SyncInfo(
                        on_wait=[waits[-1]], on_update=list(si.on_update or [])
                    )
                    changed = True
                newlist.append(ins)
            if changed:
                while len(insts):
                    insts.pop()
                for x in newlist:
                    insts.append(x)
    return n


def _build():
    nc = bass.Bass("TRN2", target_bir_lowering=False, debug=False)
    gpw = nc.dram_tensor("gpw", [SPC, PART, GPW_COLS], FP8, kind="ExternalInput")
    gte = nc.dram_tensor("gte", [SPC, PART, GTE_COLS], FP8, kind="ExternalInput")
    # ce = [ eye(128) | sel16 | ones16 ]:
    #   sel16[k, m] = 1 iff k == P  (broadcasts acc's st row to partitions 0..15)
    #   ones16[k] = 1 iff k < 16    (final score reduction column)
    ce = nc.dram_tensor("ce", [PART, PART + 17], F32, kind="ExternalInput")
    y = nc.dram_tensor("y", [1, SPC], F32, kind="ExternalOutput")

    with tile.TileContext(nc) as tc:
        with (
            tc.tile_pool(name="big", bufs=2) as big,
            tc.tile_pool(name="scratch", bufs=2) as scratch,
            tc.tile_pool(name="small", bufs=2) as small,
            tc.tile_pool(name="singles", bufs=1) as singles,
            tc.tile_pool(name="psmain", bufs=2, space="PSUM") as psmain,
            tc.tile_pool(name="pswarm", bufs=1, space="PSUM") as pswarm,
            tc.tile_pool(name="psaux", bufs=1, space="PSUM") as psaux,
        ):
            e_sb = singles.tile([PART, PART + 17], F32)
            out_sb = singles.tile([1, SPC], F32)
            lnbias = singles.tile([PART, 1], F32)

            gpws = [big.tile([PART, GPW_COLS], FP8, tag="gpw", name=f"gpw{s}")
                    for s in range(SPC)]
            gtes = [big.tile([PART, GTE_COLS], FP8, tag="gte", name=f"gte{s}")
                    for s in range(SPC)]

            # ---- input DMAs: plain HWDGE, 128 contiguous descriptors each,
            # split across both hwdge queues (sync issues ~650ns/DMA).
            # Arrival order: gpw0, gpw1 (halves in parallel on the 2 queues),
            # then gte pieces interleaved s0/s1 so both main GEMMs chase the
            # tail of the DMA stream. ----
            # gpw0 quartered (2 per queue) so den0 pipelines behind arrival;
            # gpw1/gte halved across the queues.  12 DMAs total; early ones
            # complete before their semaphore lanes recycle.  high_priority
            # hoists the first DMAs ahead of the engines' register init.
            GQ = GPW_COLS // 4
            with tc.high_priority():
                nc.gpsimd.memset(lnbias[:, :], -2.0)
                nc.scalar.dma_start(out=e_sb[:, :], in_=ce[:, :])
                for q in range(4):
                    eng = nc.sync if q % 2 == 0 else nc.scalar
                    eng.dma_start(
                        out=gpws[0][:, q * GQ : (q + 1) * GQ],
                        in_=gpw[0, :, q * GQ : (q + 1) * GQ],
                    )
            GH = GPW_COLS // 2
            GTH = (GTE_CUTS[1]) * XCH  # piece 1 boundary (20 chunks)
            nc.sync.dma_start(out=gpws[1][:, 0:GH], in_=gpw[1, :, 0:GH])
            nc.scalar.dma_start(
                out=gpws[1][:, GH:GPW_COLS], in_=gpw[1, :, GH:GPW_COLS]
            )
            nc.sync.dma_start(out=gtes[0][:, 0:GTH], in_=gte[0, :, 0:GTH])
            nc.scalar.dma_start(
                out=gtes[0][:, GTH:GTE_COLS], in_=gte[0, :, GTH:GTE_COLS]
            )
            nc.sync.dma_start(out=gtes[1][:, 0:GTH], in_=gte[1, :, 0:GTH])
            nc.scalar.dma_start(
                out=gtes[1][:, GTH:GTE_COLS], in_=gte[1, :, GTH:GTE_COLS]
            )

            # ---- PE warmup: HAM releases the clock gate after ~3.4us of
            # sustained activity.  Gate dummy matmuls on the arriving gpw
            # pieces so the array stays busy through the whole DMA phase and
            # the main GEMMs run at 2.4 GHz.  Results are never read. ----
            warm_ps = pswarm.tile([MCH, 512], F32)
            for q in range(4):
                for k in range(3):
                    base = q * GQ
                    nc.tensor.matmul(
                        warm_ps[:, :],
                        gpws[0][:, base : base + MCH],
                        gpws[0][:, base + MCH + k * 512 : base + MCH + (k + 1) * 512],
                    )
            for half in range(2):
                for k in range(2):
                    base = half * GH
                    nc.tensor.matmul(
                        warm_ps[:, :],
                        gpws[1][:, base : base + MCH],
                        gpws[1][:, base + MCH + k * 512 : base + MCH + (k + 1) * 512],
                    )

            # ---- den/r per sample (DVE): den' = sum over ALL 17 p' (incl
            # ones row) = den+1, contiguous innermost reduce, one per gpw
            # half; den = max(den'-1, 1) fused; r = 1/den approx (fp32);
            # two-term fp8 split r = r_hi + r_lo written into gte r slots. ----
            rparts = {}
            denps = {}

            def den_reduce(s, frm, to):
                # DVE: den'[k,(c,js)] = sum over all 17 p' (contiguous
                # innermost) = den+1, for j in [frm, to)
                if s not in denps:
                    denps[s] = scratch.tile(
                        [PART, JW], BF16, tag="denp", name=f"denp{s}"
                    )
                gv = gpws[s][:].rearrange("part (cj p) -> part cj p", p=PP)
                with nc.allow_low_precision(
                    reason="den is an integer <=17; exact in bf16"
                ):
                    nc.vector.tensor_reduce(
                        out=denps[s][:, frm:to],
                        in_=gv[:, frm:to, :],
                        axis=mybir.AxisListType.X,
                        op=ADD,
                    )

            def r_chain(s, h):
                # r = 1/max(den'-1, 1) via exp(-ln(.)) on the ACT engine
                # (table err ~1e-3 << 2e-2; bass bans the Reciprocal table)
                if s not in rparts:
                    rparts[s] = (
                        scratch.tile([PART, JW], F32, tag="lnv", name=f"lnv{s}"),
                        scratch.tile([PART, JW], F32, tag="r32", name=f"r32{s}"),
                        scratch.tile([PART, JW], F32, tag="den32", name=f"den32{s}"),
                    )
                lnv, r32, den32 = rparts[s]
                HJ = JW // 2
                sl = slice(h * HJ, (h + 1) * HJ)
                # max(den'-1, 1) = Relu(den'-2) + 1, all on the ACT engine
                nc.scalar.activation(
                    out=den32[:, sl], in_=denps[s][:, sl],
                    func=mybir.ActivationFunctionType.Relu,
                    bias=lnbias[:, :],
                )
                nc.scalar.activation(
                    out=lnv[:, sl], in_=den32[:, sl],
                    func=mybir.ActivationFunctionType.Ln,
                    bias=1.0,
                )
                nc.scalar.activation(
                    out=r32[:, sl], in_=lnv[:, sl],
                    func=mybir.ActivationFunctionType.Exp, scale=-1.0,
                )

            # r scatter into the gte r slots (after that piece's DMA landed)
            def scatter_r(s, piece):
                r32 = rparts[s][1]
                clo, chi = GTE_CUTS[piece], GTE_CUTS[piece + 1]
                rv = gtes[s][:].rearrange("part (c x) -> part c x", x=XCH)[
                    :, clo:chi, UCOLS:XCH
                ].rearrange("part c (js two) -> part c js two", two=2)
                rsrc = r32[:].rearrange(
                    "part (c js one) -> part c js one", js=JS, one=1
                )[:, clo:chi, :, :]
                with nc.allow_low_precision(reason="fp8 r split scatter"):
                    nc.gpsimd.tensor_copy(rv[:, :, :, 0:1], rsrc)
                    nc.gpsimd.tensor_tensor(
                        rv[:, :, :, 1:2], rsrc, rv[:, :, :, 0:1],
                        mybir.AluOpType.subtract,
                    )

            # ---- main GEMM: DoubleRow fp8, 2 j-chunks per matmul, issued
            # piece-by-piece chasing the gte DMA stream ----
            pss = {
                s: psmain.tile([MCH, XCH], F32, tag="main", name=f"ps_main{s}")
                for s in range(SPC)
            }

            def main_piece(s, piece):
                lo2, hi2 = GTE_CUTS[piece] // 2, GTE_CUTS[piece + 1] // 2
                gpv = gpws[s][:].rearrange(
                    "part (c2 two m) -> part c2 two m", two=2, m=MCH
                )
                gtv = gtes[s][:].rearrange(
                    "part (c2 two x) -> part c2 two x", two=2, x=XCH
                )
                for c2 in range(lo2, hi2):
                    if USE_DR:
                        nc.tensor.matmul(
                            pss[s][:, :],
                            gpv[:, c2],
                            gtv[:, c2],
                            start=(c2 == 0),
                            stop=(c2 == NPAIR - 1),
                            perf_mode=mybir.MatmulPerfMode.DoubleRow,
                        )
                    else:
                        for c in (2 * c2, 2 * c2 + 1):
                            nc.tensor.matmul(
                                pss[s][:, :],
                                gpws[s][:, c * MCH : (c + 1) * MCH],
                                gtes[s][:, c * XCH : (c + 1) * XCH],
                                start=(c == 0),
                                stop=(c == NCH - 1),
                            )

            accs = {}

            def extract(s):
                ext = small.tile([MCH, XCH], F32, tag="ext", name=f"ext{s}")
                nc.vector.tensor_copy(ext[:, :], pss[s][:, :])
                ps_acc = psaux.tile([PP, UA + 2], F32, tag="acc", name=f"ps_acc{s}")
                for js in range(JS):
                    nc.tensor.matmul(
                        ps_acc[:, 0:UA],
                        e_sb[0:MCH, js * PP : (js + 1) * PP],
                        ext[:, js * UA : (js + 1) * UA],
                        start=(js == 0),
                        stop=(js == JS - 1),
                    )
                for js in range(JS):
                    nc.tensor.matmul(
                        ps_acc[:, UA : UA + 2],
                        e_sb[0:MCH, js * PP : (js + 1) * PP],
                        ext[:, UCOLS + js * 2 : UCOLS + js * 2 + 2],
                        start=(js == 0),
                        stop=(js == JS - 1),
                    )
                acc = small.tile([PP, UA + 2], F32, tag="accsb", name=f"acc{s}")
                nc.vector.tensor_copy(acc[:, :], ps_acc[:, :])
                accs[s] = acc

            def finish(s):
                acc = accs[s]
                # broadcast st (ones row of acc) to 16 partitions via sel16
                ps_st16 = psaux.tile([16, T], F32, tag="st16", name=f"ps_st16{s}")
                nc.tensor.matmul(
                    ps_st16[:, :], e_sb[0:PP, PART : PART + 16], acc[0:PP, 0:T]
                )
                # unions = max((st16 + sp) - inters, 1);  iou = inters/unions
                unions = small.tile([16, T], F32, tag=f"un{s}", name=f"unions{s}")
                nc.vector.scalar_tensor_tensor(
                    out=unions[:, :],
                    in0=ps_st16[:, :],
                    scalar=acc[0:P, T : T + 1],
                    in1=acc[0:P, 0:T],
                    op0=ADD,
                    op1=mybir.AluOpType.subtract,
                )
                nc.vector.tensor_scalar_max(
                    out=unions[:, :], in0=unions[:, :], scalar1=1.0
                )
                uln = small.tile([16, T], F32, tag=f"ul{s}", name=f"uln{s}")
                nc.scalar.activation(
                    out=uln[:, :], in_=unions[:, :],
                    func=mybir.ActivationFunctionType.Ln,
                )
                uinv = small.tile([16, T], F32, tag=f"ui{s}", name=f"uinv{s}")
                nc.scalar.activation(
                    out=uinv[:, :], in_=uln[:, :],
                    func=mybir.ActivationFunctionType.Exp, scale=-1.0,
                )
                iou = small.tile([16, T], F32, tag=f"iou{s}", name=f"iou{s}")
                nc.vector.tensor_tensor(
                    iou[:, :], acc[0:P, 0:T], uinv[:, :], mybir.AluOpType.mult
                )
                wmax = small.tile([16, 1], F32, tag=f"wm{s}", name=f"wmax{s}")
                nc.vector.tensor_reduce(
                    out=wmax[:, :],
                    in_=iou[:, :],
                    axis=mybir.AxisListType.X,
                    op=mybir.AluOpType.max,
                )
                # ws = (S_hi + S_lo) * w
                ws = small.tile([16, 1], F32, tag=f"ws{s}", name=f"ws{s}")
                nc.vector.scalar_tensor_tensor(
                    out=ws[:, :],
                    in0=acc[0:P, UA : UA + 1],
                    scalar=acc[0:P, UA + 1 : UA + 2],
                    in1=wmax[:, :],
                    op0=ADD,
                    op1=mybir.AluOpType.mult,
                )
                ps_score = psaux.tile([1, 1], F32, tag="sc", name=f"ps_score{s}")
                nc.tensor.matmul(
                    ps_score[:, :], e_sb[0:16, PART + 16 : PART + 17], ws[:, :]
                )
                nc.vector.tensor_scalar_mul(
                    out=out_sb[0:1, s : s + 1], in0=ps_score[:, :], scalar1=INV_HW
                )

            QJ = JW // 4
            HJ = JW // 2
            # s0 half1 (quarters), chain, scatter piece0 -> main0 p0 early
            den_reduce(0, 0, QJ)
            den_reduce(0, QJ, 2 * QJ)
            r_chain(0, 0)
            scatter_r(0, 0)
            main_piece(0, 0)
            # ALL of s1's den next, so main1 + its extraction + epilogue
            # complete underneath s0's second-half mains (tail overlap)
            den_reduce(1, 0, HJ)
            den_reduce(1, HJ, JW)
            r_chain(1, 0)
            r_chain(1, 1)
            scatter_r(1, 0)
            main_piece(1, 0)
            scatter_r(1, 1)
            scatter_r(1, 2)
            main_piece(1, 1)
            main_piece(1, 2)
            # s0 half2 (reduces emitted before extract(1) so the DVE FIFO
            # cannot block them behind the ext copy)
            den_reduce(0, 2 * QJ, 3 * QJ)
            den_reduce(0, 3 * QJ, JW)
            r_chain(0, 1)
            scatter_r(0, 1)
            scatter_r(0, 2)
            main_piece(0, 1)
            main_piece(0, 2)
            extract(1)
            finish(1)
            extract(0)
            finish(0)

            nc.sync.dma_start(out=y[:, :], in_=out_sb[:, :])

    _split_multi_waits(nc)
    return nc


_NC = None


def _get_nc():
    global _NC
    if _NC is None:
        _NC = _build()
    return _NC


def _pack(groups_pred: np.ndarray, groups_true: np.ndarray):
    # binarize (match torch .bool(): nonzero -> 1); {0,1} exact in fp8e4m3
    gp = (groups_pred != 0).astype(NPF8)     # (N, P, H, W)
    gt = (groups_true != 0).astype(NPF8)     # (N, T, H, W)
    # (N, P, PART, NCH, JS) -> (N, PART, NCH, JS, P) + ones row
    gp5 = gp.reshape(N, P, PART, NCH, JS).transpose(0, 2, 3, 4, 1)
    gpw = np.empty((N, PART, NCH, JS, PP), dtype=NPF8)
    gpw[..., 0:P] = gp5
    gpw[..., P] = NPF8(1.0)
    gt5 = gt.reshape(N, T, PART, NCH, JS).transpose(0, 2, 3, 4, 1)
    gte = np.zeros((N, PART, NCH, XCH), dtype=NPF8)
    upart = np.empty((N, PART, NCH, JS, UA), dtype=NPF8)
    upart[..., 0:T] = gt5
    upart[..., T] = NPF8(1.0)
    gte[..., 0:UCOLS] = upart.reshape(N, PART, NCH, UCOLS)
    gpw = np.ascontiguousarray(gpw.reshape(NCORES, SPC, PART, GPW_COLS))
    gte = np.ascontiguousarray(gte.reshape(NCORES, SPC, PART, GTE_COLS))
    return gpw, gte


def make_in_maps(groups_pred: np.ndarray, groups_true: np.ndarray) -> list[dict]:
    gpw, gte = _pack(groups_pred, groups_true)
    ce = np.zeros((PART, PART + 17), dtype=np.float32)
    ce[:, 0:PART] = np.eye(PART, dtype=np.float32)
    ce[P, PART : PART + 16] = 1.0
    ce[0:16, PART + 16] = 1.0
    return [{"gpw": gpw[c], "gte": gte[c], "ce": ce} for c in range(NCORES)]


def kernel(groups_pred: np.ndarray, groups_true: np.ndarray) -> np.ndarray:
    assert groups_pred.shape == (N, P, H, W)
    assert groups_true.shape == (N, T, H, W)
    in_maps = make_in_maps(groups_pred, groups_true)
    res = run_bass_kernel_spmd(_get_nc(), in_maps, core_ids=list(range(NCORES)))
    out = np.empty((N,), dtype=np.float32)
    for c in range(NCORES):
        out[c * SPC : (c + 1) * SPC] = res.results[c]["y"][0]
    return out


# revision 27
# speedup vs baseline: 1.1957x; 1.0406x over previous
"""Trainium2 Bass kernel for nn_CholecFixScore (pairwise-IoU mask scoring).

Math (per sample n):
    Gp (P=16, HW) and Gt (T=8, HW) are binary {0,1} masks.
    inters[p,t] = sum_hw Gp[p]*Gt[t];  sp[p] = sum Gp[p];  st[t] = sum Gt[t]
    iou = inters / max(sp+st-inters, 1)            (union==0 => inters==0 => iou 0)
    w[p] = max_t iou[p,t]
    den[hw] = sum_p Gp[p,hw];  r = 1/max(den,1)    (den==0 pixels have Gp==0)
    score[n] = (1/HW) * sum_p w[p] * S[p],  S[p] = sum_hw Gp[p,hw]*r[hw]
which equals the reference's mean over pixels of (sum_p w[p]Gp[p,hw])/den[hw].

Sharding: pure data parallel, 2 samples per core on 8 cores.

Host-side packing (free wrt HW time): masks are {0,1}, EXACT in fp8e4m3,
which halves HBM traffic again vs bf16 and enables DoubleRow matmuls.
Pixel index hw = k*392 + j with k = SBUF partition (128), j in [0,392).
j is chunked as j = c*7 + js  (c in [0,56), js in [0,7)).
  gpw[s] (128, 56*7*17) fp8, free = (c, js, p'):  p' = 16 Gp masks | ones row
  gte[s] (128, 56*77)  fp8, free = (c, x): x = (js,u) u = 8 Gt | ones (63 cols)
                                         then (js, r_hi|r_lo)     (14 cols)
The ones ROW (p'=16) makes the main GEMM emit rhs column sums == st[t];
the ones COLUMN (u=8) emits sp[p].  r is computed on-chip and written as
an exact-ish two-term fp8 split r = r_hi + r_lo (rel err ~2^-8 << 2e-2).

Main GEMM per sample: 56 accumulating fp8 matmuls with fully contiguous
lhsT (119 cols) / rhs (77 cols) APs into psum (119, 77).  Valid outputs
are the 7 diagonal js-blocks; eye-selector matmuls relocate+sum them
into acc (17, 11) = [inters|sp|S_hi|S_lo ; st|...].  (DoubleRow fails
this walrus's ISA check for every shape, so plain fp8 matmuls.)

den'[k,j] = sum_p' gpw (all 17 p', contiguous innermost DVE reduce, one
per gpw DMA piece) = den+1; the whole r chain runs on the ACT engine:
max(den'-1,1) = Relu(den'-2)+1, then 1/den = exp(-ln(den)) (table err
~1e-3; bass bans the Reciprocal table).  The fp8 two-term r split is
scattered straight into the gte r slots by gpsimd (copy + subtract).
All DMAs are plain HWDGE with 128 contiguous descriptors, split over
BOTH hwdge queues (sync + scalar: one queue issues only ~1.5 DMA/us),
ordered gpw0 (quartered), gpw1, gte0, gte1 so den/r for each sample
completes while later DMAs stream, and each main GEMM chases its gte
pieces.  Dummy matmuls reading each arrived gpw piece keep the PE
clock at 2.4 GHz through the DMA phase.
"""

import numpy as np
import ml_dtypes

import concourse.bass as bass
import concourse.tile as tile
from concourse import mybir
from concourse.bass_utils import run_bass_kernel_spmd

F32 = mybir.dt.float32
BF16 = mybir.dt.bfloat16
FP8 = mybir.dt.float8e4
ADD = mybir.AluOpType.add
NPF8 = ml_dtypes.float8_e4m3fn

N, P, T = 16, 16, 8
H, W = 224, 224
HW = H * W            # 50176
PART = 128
JW = HW // PART       # 392 j values per partition
JS = 7                # j values per chunk
NCH = JW // JS        # 56 chunks
NPAIR = NCH // 2      # 28 DoubleRow chunk pairs
PP = P + 1            # 16 masks + ones row
UA = T + 1            # u-part: 8 Gt | ones
MCH = JS * PP         # 119 lhsT cols per chunk
UCOLS = JS * UA       # 63 u-part cols per chunk
RCOLS = JS * 2        # 14 r cols per chunk (js-major, hi|lo)
XCH = UCOLS + RCOLS   # 77 rhs cols per chunk
GPW_COLS = NCH * MCH  # 6664
GTE_COLS = NCH * XCH  # 4312
NCORES = 8
SPC = N // NCORES     # samples per core = 2
INV_HW = 1.0 / HW
USE_DR = False        # DoubleRow perf mode (fp8, 2 k-tiles per matmul)
# DMA pieces: gpw halves, gte thirds (chunk-pair aligned: 20|18|18 chunks)
GTE_CUTS = [0, 20, 38, NCH]
N_WARM_PER_PIECE = 5  # dummy matmuls gated on each gpw DMA piece


def _split_multi_waits(nc):
    """The pinned walrus encodes only ONE sync-wait per instruction; split
    Tile-emitted multi-wait instructions into single-wait NOPs ahead of them
    (same engine, program order => identical semantics)."""
    n = 0
    for f in nc.m.functions:
        for bb in f.blocks:
            insts = bb.instructions
            newlist = []
            changed = False
            for ins in insts:
                si = ins.sync_info
                if si is not None and si.on_wait is not None and len(si.on_wait) > 1:
                    waits = list(si.on_wait)
                    for w in waits[:-1]:
                        n += 1
                        newlist.append(
                            mybir.InstNoOp(
                                name=f"I-waitsplit-{n}",
                                engine=ins.engine,
                                ins=[],
                                outs=[],
                                sync_info=mybir.SyncInfo(on_wait=[w], on_update=[]),
                            )
                        )
                    ins.sync_info = mybir.SyncInfo(
                        on_wait=[waits[-1]], on_update=list(si.on_update or [])
                    )
                    changed = True
                newlist.append(ins)
            if changed:
                while len(insts):
                    insts.pop()
                for x in newlist:
                    insts.append(x)
    return n


def _build():
    nc = bass.Bass("TRN2", target_bir_lowering=False, debug=False)
    gpw = nc.dram_tensor("gpw", [SPC, PART, GPW_COLS], FP8, kind="ExternalInput")
    gte = nc.dram_tensor("gte", [SPC, PART, GTE_COLS], FP8, kind="ExternalInput")
    # ce = [ eye(128) | sel16 | ones16 ]:
    #   sel16[k, m] = 1 iff k == P  (broadcasts acc's st row to partitions 0..15)
    #   ones16[k] = 1 iff k < 16    (final score reduction column)
    ce = nc.dram_tensor("ce", [PART, PART + 17], F32, kind="ExternalInput")
    y = nc.dram_tensor("y", [1, SPC], F32, kind="ExternalOutput")

    with tile.TileContext(nc) as tc:
        with (
            tc.tile_pool(name="big", bufs=2) as big,
            tc.tile_pool(name="scratch", bufs=2) as scratch,
            tc.tile_pool(name="small", bufs=2) as small,
            tc.tile_pool(name="singles", bufs=1) as singles,
            tc.tile_pool(name="psmain", bufs=2, space="PSUM") as psmain,
            tc.tile_pool(name="pswarm", bufs=1, space="PSUM") as pswarm,
            tc.tile_pool(name="psaux", bufs=1, space="PSUM") as psaux,
        ):
            e_sb = singles.tile([PART, PART + 17], F32)
            out_sb = singles.tile([1, SPC], F32)
            lnbias = singles.tile([PART, 1], F32)

            gpws = [big.tile([PART, GPW_COLS], FP8, tag="gpw", name=f"gpw{s}")
                    for s in range(SPC)]
            gtes = [big.tile([PART, GTE_COLS], FP8, tag="gte", name=f"gte{s}")
                    for s in range(SPC)]

            # ---- input DMAs: plain HWDGE, 128 contiguous descriptors each,
            # split across both hwdge queues (sync issues ~650ns/DMA).
            # Arrival order: gpw0, gpw1 (halves in parallel on the 2 queues),
            # then gte pieces interleaved s0/s1 so both main GEMMs chase the
            # tail of the DMA stream. ----
            # gpw0 quartered (2 per queue) so den0 pipelines behind arrival;
            # gpw1/gte halved across the queues.  12 DMAs total; early ones
            # complete before their semaphore lanes recycle.  high_priority
            # hoists the first DMAs ahead of the engines' register init.
            GQ = GPW_COLS // 4
            with tc.high_priority():
                nc.gpsimd.memset(lnbias[:, :], -2.0)
                nc.scalar.dma_start(out=e_sb[:, :], in_=ce[:, :])
                for q in range(4):
                    eng = nc.sync if q % 2 == 0 else nc.scalar
                    eng.dma_start(
                        out=gpws[0][:, q * GQ : (q + 1) * GQ],
                        in_=gpw[0, :, q * GQ : (q + 1) * GQ],
                    )
            GH = GPW_COLS // 2
            GTH = (GTE_CUTS[1]) * XCH  # piece 1 boundary (20 chunks)
            nc.sync.dma_start(out=gpws[1][:, 0:GH], in_=gpw[1, :, 0:GH])
            nc.scalar.dma_start(
                out=gpws[1][:, GH:GPW_COLS], in_=gpw[1, :, GH:GPW_COLS]
            )
            nc.sync.dma_start(out=gtes[0][:, 0:GTH], in_=gte[0, :, 0:GTH])
            nc.scalar.dma_start(
                out=gtes[0][:, GTH:GTE_COLS], in_=gte[0, :, GTH:GTE_COLS]
            )
            nc.sync.dma_start(out=gtes[1][:, 0:GTH], in_=gte[1, :, 0:GTH])
            nc.scalar.dma_start(
                out=gtes[1][:, GTH:GTE_COLS], in_=gte[1, :, GTH:GTE_COLS]
            )

            # ---- PE warmup: HAM releases the clock gate after ~3.4us of
            # sustained activity.  Gate dummy matmuls on the arriving gpw
            # pieces so the array stays busy through the whole DMA phase and
            # the main GEMMs run at 2.4 GHz.  Results are never read. ----
            warm_ps = pswarm.tile([MCH, 512], F32)
            for q in range(4):
                for k in range(3):
                    base = q * GQ
                    nc.tensor.matmul(
                        warm_ps[:, :],
                        gpws[0][:, base : base + MCH],
                        gpws[0][:, base + MCH + k * 512 : base + MCH + (k + 1) * 512],
                    )
            for half in range(2):
                for k in range(2):
                    base = half * GH
                    nc.tensor.matmul(
                        warm_ps[:, :],
                        gpws[1][:, base : base + MCH],
                        gpws[1][:, base + MCH + k * 512 : base + MCH + (k + 1) * 512],
                    )

            # ---- den/r per sample (DVE): den' = sum over ALL 17 p' (incl
            # ones row) = den+1, contiguous innermost reduce, one per gpw
            # half; den = max(den'-1, 1) fused; r = 1/den approx (fp32);
            # two-term fp8 split r = r_hi + r_lo written into gte r slots. ----
            rparts = {}
            denps = {}

            def den_reduce(s, frm, to):
                # DVE: den'[k,(c,js)] = sum over all 17 p' (contiguous
                # innermost) = den+1, for j in [frm, to)
                if s not in denps:
                    denps[s] = scratch.tile(
                        [PART, JW], BF16, tag="denp", name=f"denp{s}"
                    )
                gv = gpws[s][:].rearrange("part (cj p) -> part cj p", p=PP)
                with nc.allow_low_precision(
                    reason="den is an integer <=17; exact in bf16"
                ):
                    nc.vector.tensor_reduce(
                        out=denps[s][:, frm:to],
                        in_=gv[:, frm:to, :],
                        axis=mybir.AxisListType.X,
                        op=ADD,
                    )

            def r_chain(s, h):
                # r = 1/max(den'-1, 1) via exp(-ln(.)) on the ACT engine
                # (table err ~1e-3 << 2e-2; bass bans the Reciprocal table)
                if s not in rparts:
                    rparts[s] = (
                        scratch.tile([PART, JW], F32, tag="lnv", name=f"lnv{s}"),
                        scratch.tile([PART, JW], F32, tag="r32", name=f"r32{s}"),
                        scratch.tile([PART, JW], F32, tag="den32", name=f"den32{s}"),
                    )
                lnv, r32, den32 = rparts[s]
                HJ = JW // 2
                sl = slice(h * HJ, (h + 1) * HJ)
                # max(den'-1, 1) = Relu(den'-2) + 1, all on the ACT engine
                nc.scalar.activation(
                    out=den32[:, sl], in_=denps[s][:, sl],
                    func=mybir.ActivationFunctionType.Relu,
                    bias=lnbias[:, :],
                )
                nc.scalar.activation(
                    out=lnv[:, sl], in_=den32[:, sl],
                    func=mybir.ActivationFunctionType.Ln,
                    bias=1.0,
                )
                nc.scalar.activation(
                    out=r32[:, sl], in_=lnv[:, sl],
                    func=mybir.ActivationFunctionType.Exp, scale=-1.0,
                )

            # r scatter into the gte r slots (after that piece's DMA landed)
            def scatter_r(s, piece):
                r32 = rparts[s][1]
                clo, chi = GTE_CUTS[piece], GTE_CUTS[piece + 1]
                rv = gtes[s][:].rearrange("part (c x) -> part c x", x=XCH)[
                    :, clo:chi, UCOLS:XCH
                ].rearrange("part c (js two) -> part c js two", two=2)
                rsrc = r32[:].rearrange(
                    "part (c js one) -> part c js one", js=JS, one=1
                )[:, clo:chi, :, :]
                with nc.allow_low_precision(reason="fp8 r split scatter"):
                    nc.gpsimd.tensor_copy(rv[:, :, :, 0:1], rsrc)
                    nc.gpsimd.tensor_tensor(
                        rv[:, :, :, 1:2], rsrc, rv[:, :, :, 0:1],
                        mybir.AluOpType.subtract,
                    )

            # ---- main GEMM: DoubleRow fp8, 2 j-chunks per matmul, issued
            # piece-by-piece chasing the gte DMA stream ----
            pss = {
                s: psmain.tile([MCH, XCH], F32, tag="main", name=f"ps_main{s}")
                for s in range(SPC)
            }

            def main_piece(s, piece):
                lo2, hi2 = GTE_CUTS[piece] // 2, GTE_CUTS[piece + 1] // 2
                gpv = gpws[s][:].rearrange(
                    "part (c2 two m) -> part c2 two m", two=2, m=MCH
                )
                gtv = gtes[s][:].rearrange(
                    "part (c2 two x) -> part c2 two x", two=2, x=XCH
                )
                for c2 in range(lo2, hi2):
                    if USE_DR:
                        nc.tensor.matmul(
                            pss[s][:, :],
                            gpv[:, c2],
                            gtv[:, c2],
                            start=(c2 == 0),
                            stop=(c2 == NPAIR - 1),
                            perf_mode=mybir.MatmulPerfMode.DoubleRow,
                        )
                    else:
                        for c in (2 * c2, 2 * c2 + 1):
                            nc.tensor.matmul(
                                pss[s][:, :],
                                gpws[s][:, c * MCH : (c + 1) * MCH],
                                gtes[s][:, c * XCH : (c + 1) * XCH],
                                start=(c == 0),
                                stop=(c == NCH - 1),
                            )

            accs = {}

            def extract(s):
                ext = small.tile([MCH, XCH], F32, tag="ext", name=f"ext{s}")
                nc.vector.tensor_copy(ext[:, :], pss[s][:, :])
                ps_acc = psaux.tile([PP, UA + 2], F32, tag="acc", name=f"ps_acc{s}")
                for js in range(JS):
                    nc.tensor.matmul(
                        ps_acc[:, 0:UA],
                        e_sb[0:MCH, js * PP : (js + 1) * PP],
                        ext[:, js * UA : (js + 1) * UA],
                        start=(js == 0),
                        stop=(js == JS - 1),
                    )
                for js in range(JS):
                    nc.tensor.matmul(
                        ps_acc[:, UA : UA + 2],
                        e_sb[0:MCH, js * PP : (js + 1) * PP],
                        ext[:, UCOLS + js * 2 : UCOLS + js * 2 + 2],
                        start=(js == 0),
                        stop=(js == JS - 1),
                    )
                acc = small.tile([PP, UA + 2], F32, tag="accsb", name=f"acc{s}")
                nc.vector.tensor_copy(acc[:, :], ps_acc[:, :])
                accs[s] = acc

            def finish(s):
                acc = accs[s]
                # broadcast st (ones row of acc) to 16 partitions via sel16
                ps_st16 = psaux.tile([16, T], F32, tag="st16", name=f"ps_st16{s}")
                nc.tensor.matmul(
                    ps_st16[:, :], e_sb[0:PP, PART : PART + 16], acc[0:PP, 0:T]
                )
                # unions = max((st16 + sp) - inters, 1);  iou = inters/unions
                unions = small.tile([16, T], F32, tag=f"un{s}", name=f"unions{s}")
                nc.vector.scalar_tensor_tensor(
                    out=unions[:, :],
                    in0=ps_st16[:, :],
                    scalar=acc[0:P, T : T + 1],
                    in1=acc[0:P, 0:T],
                    op0=ADD,
                    op1=mybir.AluOpType.subtract,
                )
                nc.vector.tensor_scalar_max(
                    out=unions[:, :], in0=unions[:, :], scalar1=1.0
                )
                uln = small.tile([16, T], F32, tag=f"ul{s}", name=f"uln{s}")
                nc.scalar.activation(
                    out=uln[:, :], in_=unions[:, :],
                    func=mybir.ActivationFunctionType.Ln,
                )
                uinv = small.tile([16, T], F32, tag=f"ui{s}", name=f"uinv{s}")
                nc.scalar.activation(
                    out=uinv[:, :], in_=uln[:, :],
                    func=mybir.ActivationFunctionType.Exp, scale=-1.0,
                )
                iou = small.tile([16, T], F32, tag=f"iou{s}", name=f"iou{s}")
                nc.vector.tensor_tensor(
                    iou[:, :], acc[0:P, 0:T], uinv[:, :], mybir.AluOpType.mult
                )
                wmax = small.tile([16, 1], F32, tag=f"wm{s}", name=f"wmax{s}")
                nc.vector.tensor_reduce(
                    out=wmax[:, :],
                    in_=iou[:, :],
                    axis=mybir.AxisListType.X,
                    op=mybir.AluOpType.max,
                )
                # ws = (S_hi + S_lo) * w
                ws = small.tile([16, 1], F32, tag=f"ws{s}", name=f"ws{s}")
                nc.vector.scalar_tensor_tensor(
                    out=ws[:, :],
                    in0=acc[0:P, UA : UA + 1],
                    scalar=acc[0:P, UA + 1 : UA + 2],
                    in1=wmax[:, :],
                    op0=ADD,
                    op1=mybir.AluOpType.mult,
                )
                ps_score = psaux.tile([1, 1], F32, tag="sc", name=f"ps_score{s}")
                nc.tensor.matmul(
                    ps_score[:, :], e_sb[0:16, PART + 16 : PART + 17], ws[:, :]
                )
                nc.vector.tensor_scalar_mul(
                    out=out_sb[0:1, s : s + 1], in0=ps_score[:, :], scalar1=INV_HW
                )

            QJ = JW // 4
            HJ = JW // 2
            # s0 dens first (quarters, chased): main0 drains early
            den_reduce(0, 0, QJ)
            den_reduce(0, QJ, 2 * QJ)
            r_chain(0, 0)
            scatter_r(0, 0)
            main_piece(0, 0)
            den_reduce(0, 2 * QJ, 3 * QJ)
            den_reduce(0, 3 * QJ, JW)
            r_chain(0, 1)
            scatter_r(0, 1)
            scatter_r(0, 2)
            main_piece(0, 1)
            main_piece(0, 2)
            # s1 dens right behind; s0's extraction+epilogue slot into the
            # PE gap between main1's r-halves, so only s1's tail remains
            den_reduce(1, 0, HJ)
            den_reduce(1, HJ, JW)
            r_chain(1, 0)
            r_chain(1, 1)
            scatter_r(1, 0)
            main_piece(1, 0)
            extract(0)
            finish(0)
            scatter_r(1, 1)
            scatter_r(1, 2)
            main_piece(1, 1)
            main_piece(1, 2)
            extract(1)
            finish(1)
            nc.sync.dma_start(out=y[:, :], in_=out_sb[:, :])

    _split_multi_waits(nc)
    return nc


_NC = None


def _get_nc():
    global _NC
    if _NC is None:
        _NC = _build()
    return _NC


def _pack(groups_pred: np.ndarray, groups_true: np.ndarray):
    # binarize (match torch .bool(): nonzero -> 1); {0,1} exact in fp8e4m3
    gp = (groups_pred != 0).astype(NPF8)     # (N, P, H, W)
    gt = (groups_true != 0).astype(NPF8)     # (N, T, H, W)
    # (N, P, PART, NCH, JS) -> (N, PART, NCH, JS, P) + ones row
    gp5 = gp.reshape(N, P, PART, NCH, JS).transpose(0, 2, 3, 4, 1)
    gpw = np.empty((N, PART, NCH, JS, PP), dtype=NPF8)
    gpw[..., 0:P] = gp5
    gpw[..., P] = NPF8(1.0)
    gt5 = gt.reshape(N, T, PART, NCH, JS).transpose(0, 2, 3, 4, 1)
    gte = np.zeros((N, PART, NCH, XCH), dtype=NPF8)
    upart = np.empty((N, PART, NCH, JS, UA), dtype=NPF8)
    upart[..., 0:T] = gt5
    upart[..., T] = NPF8(1.0)
    gte[..., 0:UCOLS] = upart.reshape(N, PART, NCH, UCOLS)
    gpw = np.ascontiguousarray(gpw.reshape(NCORES, SPC, PART, GPW_COLS))
    gte = np.ascontiguousarray(gte.reshape(NCORES, SPC, PART, GTE_COLS))
    return gpw, gte


def make_in_maps(groups_pred: np.ndarray, groups_true: np.ndarray) -> list[dict]:
    gpw, gte = _pack(groups_pred, groups_true)
    ce = np.zeros((PART, PART + 17), dtype=np.float32)
    ce[:, 0:PART] = np.eye(PART, dtype=np.float32)
    ce[P, PART : PART + 16] = 1.0
    ce[0:16, PART + 16] = 1.0
    return [{"gpw": gpw[c], "gte": gte[c], "ce": ce} for c in range(NCORES)]


def kernel(groups_pred: np.ndarray, groups_true: np.ndarray) -> np.ndarray:
    assert groups_pred.shape == (N, P, H, W)
    assert groups_true.shape == (N, T, H, W)
    in_maps = make_in_maps(groups_pred, groups_true)
    res = run_bass_kernel_spmd(_get_nc(), in_maps, core_ids=list(range(NCORES)))
    out = np.empty((N,), dtype=np.float32)
    for c in range(NCORES):
        out[c * SPC : (c + 1) * SPC] = res.results[c]["y"][0]
    return out
